# revision 56
# baseline (speedup 1.0000x reference)
"""GNN message-passing kernel (GCNConv + TransformerConv layer) for 8 Trainium2 cores.

V2 design (edges sharded by dst node; balanced host-side node permutation):
  * Host permutes nodes into 160 balanced groups of <=128 (snake-deal by in-degree)
    so every (core, group) has ~E/160 edges -> TG = ceil(max/128) = 16 tiles/group,
    and core shards are 2560-slot aligned (2500 real + pad), matching phase-1 blocks.
  * Phase 1: h0s = (x @ W_gcn) * dinv_src data-parallel over 512-node blocks (fp16);
    AllGather CHUNKED per block (5 AGs) so the exchange overlaps the matmuls.
    h0s rows of OWN nodes kept in SBUF (h0s_own) for the self-loop term.
  * Sweep A (GCN aggregation): per group dma_gather h0s[src] rows (no self loops),
    S-indicator built on DVE, segment-sum via PE matmul S^T @ G in PSUM; self-loop
    term added via identity matmul of h0s_own; copy-out = one scalar activation
    LeakyReLU(dinv_dst * x) (biases are zero -> exact fusion; general path kept).
  * Projections q,k,v,s per group (fp16); k|v packed -> kv_local; kv AllGather
    chunked over 5 group-ranges, interleaved with the aggregation loop.
  * Sweep B (alpha): gather kv[src] rows; QE = ST^T-matmul selects q[dst] per edge
    (ST = host-precomputed one-hot [s,e] streamed from DRAM); alpha = fused
    mult-mult-accum of QE * k on DVE. No per-tile transposes / copies.
  * Stats: tiny AllReduce of (sum, sumsq); sigmoid -> per-edge scale.
  * Sweep C: scaled S built per tile (DVE/GpSimd alternating), segment-sum of
    v[src] via PE; skip h@Ws added via identity matmul; out rows DMA'd per group.

Gathers ride 4 SWDGE queues round-robin; DMA drain (~33ns/512B desc/engine) is the
pacing resource, so everything else is spread across engines to hide behind it.
"""

from contextlib import ExitStack

import numpy as np

F16 = np.float16

# -------------------- problem constants (nn_DimEncoder_19894288515585) ------------
FULL_CFG = dict(N=20000, E=320000, F_IN=1024, H=256, D=128, C=8)
SCALE_PARAM = 3.0
LEAKY_SLOPE = 0.01


def _derive(cfg):
    N, C = cfg["N"], cfg["C"]
    d = dict(cfg)
    d["G"] = G = 20                      # groups (128-slot) per core
    d["NPC"] = NPC = G * 128             # 2560 slots per core
    d["NPAD"] = NPAD = C * NPC           # 20480 total slots
    assert NPAD >= N
    d["NBC"] = NBC = NPC // 512          # 512-row phase-1 blocks per core
    assert NBC * 512 == NPC
    d["KC"] = cfg["F_IN"] // 128
    d["HC"] = cfg["H"] // 128
    # kv AllGather chunk boundaries in groups (front-loaded, small tail).
    # Chunks 0-2 (groups < GBK[3]) form the "early" region: the first 8 tiles
    # of every group's edge list only reference srcs there, so their gathers
    # start before the tail chunks arrive.
    d["GBK"] = (0, 7, 12, 16, 19, G)
    # h0s AllGather chunks in 512-row blocks; blocks 0-2 (chunk 0) = early
    # region for the first-8-tile gathers.
    d["GBH"] = (0, 3, 5)
    return d


# -------------------- host-side preprocessing --------------------------------------

def _wrap_idx(a):
    """int16 [M] (M%16==0) -> dma_gather index layout [128, M//16]."""
    w = a.reshape(-1, 16).T.astype(np.int16)
    return np.tile(w, (8, 1))


def _balance_nodes(indeg, cfg):
    """Snake-deal nodes (sorted by in-degree desc) into C*G groups of <=128,
    then snake-deal groups into cores. Returns slot_of_node [N] (global pi-slot)."""
    N, C, G, NPC = cfg["N"], cfg["C"], cfg["G"], cfg["NPC"]
    NG = C * G
    order = np.argsort(-indeg, kind="stable")
    # group of the i-th node in sorted order: snake over NG bins
    i = np.arange(N)
    rnd, pos = i // NG, i % NG
    gbin = np.where(rnd % 2 == 0, pos, NG - 1 - pos)
    node_group = np.empty(N, np.int64)
    node_group[order] = gbin
    gload = np.bincount(node_group, weights=indeg, minlength=NG).astype(np.int64)
    # assign groups to cores: snake over sorted loads
    gorder = np.argsort(-gload, kind="stable")
    j = np.arange(NG)
    rndg, posg = j // C, j % C
    cbin = np.where(rndg % 2 == 0, posg, C - 1 - posg)
    group_core = np.empty(NG, np.int64)
    group_core[gorder] = cbin
    # local group index within core (order of appearance)
    gslot = np.full(NG, -1, np.int64)
    cnt = np.zeros(C, np.int64)
    for gid in range(NG):
        c = group_core[gid]
        gslot[gid] = cnt[c]
        cnt[c] += 1
    assert np.all(cnt == G)
    # slots within group: in node order
    slot_of_node = np.empty(N, np.int64)
    for gid in range(NG):
        nodes = np.where(node_group == gid)[0]
        assert len(nodes) <= 128
        c, gl = group_core[gid], gslot[gid]
        slot_of_node[nodes] = c * NPC + gl * 128 + np.arange(len(nodes))
    return slot_of_node


def prep_host(inputs, cfg):
    N, E, C = cfg["N"], cfg["E"], cfg["C"]
    NPC, G, NPAD, NBC = cfg["NPC"], cfg["G"], cfg["NPAD"], cfg["NBC"]
    KC, HC, F, H, D = cfg["KC"], cfg["HC"], cfg["F_IN"], cfg["H"], cfg["D"]
    GBK, GBH = cfg["GBK"], cfg["GBH"]

    x = np.asarray(inputs["x"], np.float32)
    ei = np.asarray(inputs["edge_index"])
    src, dst = ei[0].astype(np.int64), ei[1].astype(np.int64)

    indeg = np.bincount(dst, minlength=N)
    slot = _balance_nodes(indeg, cfg)

    deg = indeg + 1.0                         # + self loop
    dinv_node = 1.0 / np.sqrt(deg)
    dinv_slot = np.zeros(NPAD, np.float32)
    dinv_slot[slot] = dinv_node

    sd, ss = slot[dst], slot[src]
    # order edges by (dst group, src local-group): within each dst group, edges
    # whose src sits in the early AG regions come first -> their gather tiles
    # only touch the prefix of h0s_ext / kv_full.
    src_lg = (ss % NPC) // 128
    perm = np.lexsort((src_lg, sd // 128))
    sd, ss, src_lg = sd[perm], ss[perm], src_lg[perm]

    gcount = np.bincount(sd // 128, minlength=C * G)
    TG = max(1, int((gcount.max() + 127) // 128))
    L = TG * 128
    rp = np.zeros(C * G + 1, np.int64)
    rp[1:] = np.cumsum(gcount)

    # h0s_ext row for pi-slot p under the chunked (GBH block ranges) AG layout
    rbh = np.array(GBH) * 512
    rows_h = rbh[1:] - rbh[:-1]
    rowoff_h = np.concatenate([[0], np.cumsum(rows_h * C)])

    def h0row(p):
        pos = p % NPC
        j = np.searchsorted(rbh, pos, side="right") - 1
        return rowoff_h[j] + (p // NPC) * rows_h[j] + (pos - rbh[j])

    # kv_full row for pi-slot p under the chunked kv AllGather layout
    rb = np.array(GBK) * 128
    rows_j = rb[1:] - rb[:-1]
    rowoff = np.concatenate([[0], np.cumsum(rows_j * C)])

    def kvrow(p):
        pos = p % NPC
        j = np.searchsorted(rb, pos, side="right") - 1
        return rowoff[j] + (p // NPC) * rows_j[j] + (pos - rb[j])

    # pack edges per (core, group): seg (-1 pad), idxa (h0s row), idxkv (kv row)
    seg_p = np.full((C, G, L), -1, np.int64)
    ia_p = np.zeros((C, G, L), np.int64)
    ik_p = np.zeros((C, G, L), np.int64)
    split_ok = True
    for c in range(C):
        for g in range(G):
            gid = c * G + g
            i0, i1 = rp[gid], rp[gid + 1]
            n = i1 - i0
            seg_p[c, g, :n] = sd[i0:i1] - gid * 128
            ia_p[c, g, :n] = h0row(ss[i0:i1])
            ik_p[c, g, :n] = kvrow(ss[i0:i1])
            # first 8 tiles must only reference the early regions
            # (h0s chunk 0 = groups < GBH[1]*4; kv chunks 0-1 = groups < GBK[2])
            ncut = min(n, 8 * 128)
            lg = src_lg[i0:i0 + ncut]
            if len(lg) and (lg.max() >= GBH[1] * 4 or lg.max() >= GBK[2]):
                split_ok = False

    # ---- shared arrays
    xp = np.zeros((NPAD, F), np.float32)
    xp[slot] = x
    NB = NPAD // 512
    xt = np.ascontiguousarray(
        xp.reshape(NB, 512, KC, 128).transpose(0, 3, 2, 1)).astype(F16)

    wg = np.ascontiguousarray(
        np.asarray(inputs["W_gcn"], np.float32).reshape(KC, 128, H).transpose(1, 0, 2)
    ).astype(F16)

    def w2(name):
        w = np.asarray(inputs[name], np.float32).reshape(HC, 128, D).transpose(1, 0, 2)
        return np.ascontiguousarray(w).astype(F16)

    bias_zero = all(
        not np.any(np.asarray(inputs[b]))
        for b in ("b_gcn", "bq", "bk", "bv", "bs"))

    shared = {
        "wg": wg,
        "wq": w2("Wq"), "wk": w2("Wk"), "wv": w2("Wv"), "ws": w2("Ws"),
        "bg": np.asarray(inputs["b_gcn"], np.float32).reshape(1, H).astype(F16),
        "bq": np.asarray(inputs["bq"], np.float32).reshape(1, D).astype(F16),
        "bk": np.asarray(inputs["bk"], np.float32).reshape(1, D).astype(F16),
        "bv": np.asarray(inputs["bv"], np.float32).reshape(1, D).astype(F16),
        "bs": np.asarray(inputs["bs"], np.float32).reshape(1, D).astype(F16),
        "iota": np.tile(np.arange(128, dtype=np.float32)[None, :], (128, 1)).astype(F16),
        "identh": np.eye(128, dtype=F16),
        "ident": np.eye(128, dtype=np.float32),
        "ones": np.ones((128, 128), np.float32),
        "onesb": np.ones((1, 128), F16),
    }

    cols = np.arange(L)
    in_maps = []
    for c in range(C):
        m = dict(shared)
        m["xt"] = np.ascontiguousarray(xt[c * NBC:(c + 1) * NBC])
        m["dinv"] = dinv_slot[c * NPC:(c + 1) * NPC].reshape(G, 128).T.copy()
        m["idxa"] = np.concatenate([_wrap_idx(ia_p[c, g]) for g in range(G)], 1)
        m["idxkv"] = np.concatenate([_wrap_idx(ik_p[c, g]) for g in range(G)], 1)
        segc = seg_p[c].reshape(G, TG, 128).transpose(2, 0, 1).reshape(128, G * TG)
        m["sega"] = segc.astype(F16)
        m["seg32"] = segc.astype(np.float32)
        # ST one-hot [s, (g,t,e)] fp16 (sweep B: QE = ST^T @ q)
        st = np.zeros((128, G * TG * 128), F16)
        for g in range(G):
            sg = seg_p[c, g]
            valid = sg >= 0
            st[sg[valid], g * L + cols[valid]] = 1.0
        m["st"] = st
        # S one-hot e-part [e, (g,t,s)] fp16 (sweep C aggregation lhsT)
        stc = np.zeros((128, G * TG * 128), F16)
        e_in_tile = cols % 128
        tile_of = cols // 128
        for g in range(G):
            sg = seg_p[c, g]
            valid = sg >= 0
            stc[e_in_tile[valid], (g * TG + tile_of[valid]) * 128 + sg[valid]] = 1.0
        m["stc"] = stc
        in_maps.append(m)

    out_unperm = dict(slot=slot)
    return in_maps, dict(TG=TG, bias_zero=bias_zero, split_ok=split_ok), out_unperm


# -------------------- device program ----------------------------------------------

def build_program(cfg, TG, bias_zero, split_ok):
    import concourse.bacc as bacc
    import concourse.mybir as mybir
    from concourse.tile import TileContext

    dt = mybir.dt
    AF = mybir.ActivationFunctionType
    OP = mybir.AluOpType

    N, E, C = cfg["N"], cfg["E"], cfg["C"]
    NPC, G, NPAD, NBC = cfg["NPC"], cfg["G"], cfg["NPAD"], cfg["NBC"]
    KC, HC, H, D = cfg["KC"], cfg["HC"], cfg["H"], cfg["D"]
    GBK, GBH = cfg["GBK"], cfg["GBH"]
    _rb = [b * 128 for b in GBK]
    NCH = len(GBK) - 1
    _rowoff = [0]
    for j in range(NCH):
        _rowoff.append(_rowoff[-1] + (_rb[j + 1] - _rb[j]) * C)
    _rbh = [b * 512 for b in GBH]
    NCHH = len(GBH) - 1
    _rowoff_h = [0]
    for j in range(NCHH):
        _rowoff_h.append(_rowoff_h[-1] + (_rbh[j + 1] - _rbh[j]) * C)

    nc = bacc.Bacc("TRN2", target_bir_lowering=False, debug=False, num_devices=C,
                   num_swdge_queues=4)

    def din(name, shape, dtype):
        return nc.dram_tensor(name, list(shape), dtype, kind="ExternalInput").ap()

    xt = din("xt", [NBC, 128, KC, 512], dt.float16)
    wg = din("wg", [128, KC, H], dt.float16)
    wq, wk = din("wq", [128, HC, D], dt.float16), din("wk", [128, HC, D], dt.float16)
    wv, ws = din("wv", [128, HC, D], dt.float16), din("ws", [128, HC, D], dt.float16)
    bg = din("bg", [1, H], dt.float16)
    bq, bk = din("bq", [1, D], dt.float16), din("bk", [1, D], dt.float16)
    bv, bs = din("bv", [1, D], dt.float16), din("bs", [1, D], dt.float16)
    iota = din("iota", [128, 128], dt.float16)
    identh = din("identh", [128, 128], dt.float16)
    ident = din("ident", [128, 128], dt.float32)
    ones = din("ones", [128, 128], dt.float32)
    onesb = din("onesb", [1, 128], dt.float16)
    dinv = din("dinv", [128, G], dt.float32)
    idxa = din("idxa", [128, G * TG * 8], dt.int16)
    idxkv = din("idxkv", [128, G * TG * 8], dt.int16)
    sega = din("sega", [128, G * TG], dt.float16)
    seg32 = din("seg32", [128, G * TG], dt.float32)
    st_in = din("st", [128, G * TG * 128], dt.float16)
    stc_in = din("stc", [128, G * TG * 128], dt.float16)

    out_l = nc.dram_tensor("out", [NPC, D], dt.float32, kind="ExternalOutput").ap()

    h0s_loc = nc.dram_tensor("h0s_loc", [NPC, H], dt.float16).ap()
    h0s_ext = nc.dram_tensor("h0s_ext", [NPAD, H], dt.float16, addr_space="Shared").ap()
    kv_local = nc.dram_tensor("kv_local", [NPC, 2 * D], dt.float16).ap()
    kv_full = nc.dram_tensor("kv_full", [NPAD, 2 * D], dt.float16,
                             addr_space="Shared").ap()
    cc_in = nc.dram_tensor("cc_in", [1, 2], dt.float32).ap()
    cc_out = nc.dram_tensor("cc_out", [1, 2], dt.float32, addr_space="Shared").ap()

    groups = [list(range(C))]
    _gq = [0]

    def gather_tiles(out3, src_ap, idx_sb, g, t0, t1, elem):
        """Gather tiles [t0, t1) of group g into out3[:, 0:t1-t0, :]."""
        nc.gpsimd.dma_gather(
            out_ap=out3[:, 0:t1 - t0, :], in_ap=src_ap,
            idxs_ap=idx_sb[:, g * TG * 8 + t0 * 8:g * TG * 8 + t1 * 8],
            num_idxs=(t1 - t0) * 128, num_idxs_reg=(t1 - t0) * 128,
            elem_size=elem, queue_num=_gq[0])
        _gq[0] = (_gq[0] + 1) % 4

    with TileContext(nc) as tc, ExitStack() as ctx:
        cpool = ctx.enter_context(tc.tile_pool(name="consts", bufs=1))
        _cn = [0]

        def load_const(ap_in, shape, dtype):
            _cn[0] += 1
            t = cpool.tile(shape, dtype, tag=f"const{_cn[0]}")
            nc.sync.dma_start(out=t[:], in_=ap_in)
            return t

        wg_sb = load_const(wg, [128, KC, H], dt.float16)
        dinv_sb = load_const(dinv, [128, G], dt.float32)

        # persistent SBUF
        hpool = ctx.enter_context(tc.tile_pool(name="keep", bufs=1))
        h0s_own = hpool.tile([128, G, H], dt.float16)     # own h0s rows (self loops)
        q_all = hpool.tile([128, G, D], dt.float16)
        s_all = hpool.tile([128, G, D], dt.float16)
        apool = ctx.enter_context(tc.tile_pool(name="alpha", bufs=1))
        alpha_all = apool.tile([128, G * TG], dt.float32)
        vkeep = apool.tile([128, G * TG, D], dt.float16)

        # ================= phase 1: h0s node-block shard + chunked AllGather =======
        with tc.tile_pool(name="xt_p", bufs=2) as xt_p, \
             tc.tile_pool(name="h0ps", bufs=4, space="PSUM") as h0ps:
            for tb in range(NBC):
                xtile = xt_p.tile([128, KC, 512], dt.float16)
                nc.sync.dma_start(out=xtile[:], in_=xt[tb])
                for j in range(4):
                    g = tb * 4 + j
                    ph = h0ps.tile([128, H], dt.float32)
                    for k in range(KC):
                        nc.tensor.matmul(ph[:],
                                         lhsT=xtile[:, k, j * 128:(j + 1) * 128],
                                         rhs=wg_sb[:, k, :],
                                         start=(k == 0), stop=(k == KC - 1))
                    if j % 2 == 0:
                        nc.vector.tensor_scalar(out=h0s_own[:, g, :], in0=ph[:],
                                                scalar1=dinv_sb[:, g:g + 1],
                                                scalar2=None, op0=OP.mult)
                    else:
                        nc.scalar.activation(h0s_own[:, g, :], ph[:], AF.Copy,
                                             scale=dinv_sb[:, g:g + 1])
                nc.sync.dma_start(
                    out=h0s_loc[tb * 512:(tb + 1) * 512, :].rearrange(
                        "(j p) h -> p j h", p=128),
                    in_=h0s_own[:, tb * 4:(tb + 1) * 4, :])
                if (tb + 1) in GBH[1:]:
                    j = GBH[1:].index(tb + 1)
                    nc.gpsimd.collective_compute(
                        "AllGather", mybir.AluOpType.bypass, replica_groups=groups,
                        ins=[h0s_loc[_rbh[j]:_rbh[j + 1], :]],
                        outs=[h0s_ext[_rowoff_h[j]:_rowoff_h[j + 1], :]])

        # gather-phase constants (loaded after phase 1 so x tiles go first)
        iota_sb = load_const(iota, [128, 128], dt.float16)
        identh_sb = load_const(identh, [128, 128], dt.float16)
        ones_sb = load_const(ones, [128, 128], dt.float32)
        onesb_sb = load_const(onesb, [1, 128], dt.float16)
        w_sb = {n: load_const(a, [128, HC, D], dt.float16)
                for n, a in (("q", wq), ("k", wk), ("v", wv), ("s", ws))}
        idxa_sb = load_const(idxa, [128, G * TG * 8], dt.int16)
        idxkv_sb = load_const(idxkv, [128, G * TG * 8], dt.int16)
        sega_sb = load_const(sega, [128, G * TG], dt.float16)

        b_sb = bgb_sb = None
        if not bias_zero:
            b_sb = {n: load_const(a, [1, D], dt.float16)
                    for n, a in (("q", bq), ("k", bk), ("v", bv), ("s", bs))}
            bg_sb = load_const(bg, [1, H], dt.float16)
            with tc.tile_pool(name="psb", bufs=1, space="PSUM") as psb:
                pb = psb.tile([128, H], dt.float32)
                nc.tensor.matmul(pb[:], lhsT=onesb_sb[:1, :], rhs=bg_sb[:1, :],
                                 start=True, stop=True)
                bgb_sb = cpool.tile([128, H], dt.float32)
                nc.vector.tensor_copy(bgb_sb[:], pb[:])

        # ============ sweep A: GCN aggregation + projections + kv exchange =========
        TG1 = min(8, TG)            # early tiles (srcs in h0s AG chunk 0)
        h0s_early = h0s_ext[0:_rowoff_h[1], :] if split_ok else h0s_ext
        with tc.tile_pool(name="gaA", bufs=13) as gaA_p, \
             tc.tile_pool(name="gaB", bufs=4) as gaB_p, \
             tc.tile_pool(name="sa", bufs=3) as sa_p, \
             tc.tile_pool(name="aps", bufs=2, space="PSUM") as aps, \
             tc.tile_pool(name="hsb", bufs=2) as hsb_p, \
             tc.tile_pool(name="ht", bufs=2) as ht_p, \
             tc.tile_pool(name="tps", bufs=2, space="PSUM") as tps, \
             tc.tile_pool(name="qps", bufs=4, space="PSUM") as qps, \
             tc.tile_pool(name="stg", bufs=2) as stg:
            gaA_t = []
            for g in range(G):
                gaA = gaA_p.tile([128, TG1, H], dt.float16)
                gather_tiles(gaA, h0s_early, idxa_sb, g, 0, TG1, H)
                gaA_t.append(gaA)
            for g in range(G):
                gaB = None
                if TG > TG1:
                    gaB = gaB_p.tile([128, TG - TG1, H], dt.float16)
                    gather_tiles(gaB, h0s_ext, idxa_sb, g, TG1, TG, H)
                gaA = gaA_t[g]
                sg = sa_p.tile([128, TG, 128], dt.float16)
                nc.vector.tensor_tensor(
                    out=sg[:],
                    in0=iota_sb[:].unsqueeze(1).broadcast_to([128, TG, 128]),
                    in1=sega_sb[:, g * TG:(g + 1) * TG].unsqueeze(2)
                        .broadcast_to([128, TG, 128]),
                    op=OP.is_equal)
                ph = aps.tile([128, H], dt.float32)
                for t in range(TG):
                    ga_ap = gaA[:, t, :] if t < TG1 else gaB[:, t - TG1, :]
                    nc.tensor.matmul(ph[:], lhsT=sg[:, t, :], rhs=ga_ap,
                                     start=(t == 0), stop=False)
                # + self-loop term via identity matmul of own h0s rows
                nc.tensor.matmul(ph[:], lhsT=identh_sb[:], rhs=h0s_own[:, g, :],
                                 start=False, stop=True)
                h16 = hsb_p.tile([128, H], dt.float16)
                if bias_zero:
                    # LeakyReLU(dinv*x) == dinv*LeakyReLU(x), dinv > 0
                    nc.scalar.activation(h16[:], ph[:], AF.Lrelu,
                                         scale=dinv_sb[:, g:g + 1],
                                         alpha=LEAKY_SLOPE)
                else:
                    hf = hsb_p.tile([128, H], dt.float32, tag="hf")
                    nc.vector.tensor_scalar(out=hf[:], in0=ph[:],
                                            scalar1=dinv_sb[:, g:g + 1],
                                            scalar2=None, op0=OP.mult)
                    nc.vector.tensor_tensor(out=hf[:], in0=hf[:], in1=bgb_sb[:],
                                            op=OP.add)
                    nc.scalar.activation(h16[:], hf[:], AF.Lrelu, alpha=LEAKY_SLOPE)
                # ---- layer-2 projections for this group
                ht = ht_p.tile([128, HC, 128], dt.float16)
                for hc in range(HC):
                    pt = tps.tile([128, 128], dt.float16)
                    nc.tensor.transpose(pt[:], h16[:, hc * 128:(hc + 1) * 128],
                                        identh_sb[:])
                    eng = (nc.vector, nc.scalar)[hc % 2]
                    if hc % 2 == 0:
                        nc.vector.tensor_copy(ht[:, hc, :], pt[:])
                    else:
                        nc.scalar.activation(ht[:, hc, :], pt[:], AF.Copy)
                kv_st = stg.tile([128, 2, D], dt.float16, tag="kv_st")
                for i, name in enumerate(("q", "k", "v", "s")):
                    pq = qps.tile([128, D], dt.float32)
                    for hc in range(HC):
                        last = (hc == HC - 1) and bias_zero
                        nc.tensor.matmul(pq[:], lhsT=ht[:, hc, :],
                                         rhs=w_sb[name][:, hc, :],
                                         start=(hc == 0), stop=last)
                    if not bias_zero:
                        nc.tensor.matmul(pq[:], lhsT=onesb_sb[:1, :],
                                         rhs=b_sb[name][:1, :],
                                         start=False, stop=True)
                    dst_ap = {"q": q_all[:, g, :], "k": kv_st[:, 0, :],
                              "v": kv_st[:, 1, :], "s": s_all[:, g, :]}[name]
                    if i % 2 == 0:
                        nc.vector.tensor_copy(dst_ap, pq[:])
                    else:
                        nc.scalar.activation(dst_ap, pq[:], AF.Copy)
                nc.sync.dma_start(out=kv_local[g * 128:(g + 1) * 128, :],
                                  in_=kv_st[:].rearrange("p a b -> p (a b)"))
                if (g + 1) in GBK[1:]:
                    j = GBK[1:].index(g + 1)
                    nc.gpsimd.collective_compute(
                        "AllGather", mybir.AluOpType.bypass, replica_groups=groups,
                        ins=[kv_local[_rb[j]:_rb[j + 1], :]],
                        outs=[kv_full[_rowoff[j]:_rowoff[j + 1], :]])

        # ================= sweep B: alpha ==========================================
        kv_early = kv_full[0:_rowoff[2], :] if split_ok else kv_full
        with tc.tile_pool(name="kgA", bufs=13) as kgA_p, \
             tc.tile_pool(name="kgB", bufs=4) as kgB_p, \
             tc.tile_pool(name="stt", bufs=3) as st_p, \
             tc.tile_pool(name="bps", bufs=4, space="PSUM") as bps, \
             tc.tile_pool(name="scr", bufs=3) as scr_p:
            kgA_t = []
            for g in range(G):
                kgA = kgA_p.tile([128, TG1, 2 * D], dt.float16)
                gather_tiles(kgA, kv_early, idxkv_sb, g, 0, TG1, 2 * D)
                kgA_t.append(kgA)
            for g in range(G):
                kgB = None
                if TG > TG1:
                    kgB = kgB_p.tile([128, TG - TG1, 2 * D], dt.float16)
                    gather_tiles(kgB, kv_full, idxkv_sb, g, TG1, TG, 2 * D)
                kgA = kgA_t[g]
                stg_t = st_p.tile([128, TG, 128], dt.float16)
                nc.sync.dma_start(
                    out=stg_t[:],
                    in_=st_in[:, g * TG * 128:(g + 1) * TG * 128].rearrange(
                        "p (t e) -> p t e", t=TG))
                if g % 2 == 0:
                    nc.vector.tensor_copy(vkeep[:, g * TG:g * TG + TG1, :],
                                          kgA[:, :, D:2 * D])
                    if kgB is not None:
                        nc.vector.tensor_copy(vkeep[:, g * TG + TG1:(g + 1) * TG, :],
                                              kgB[:, :, D:2 * D])
                else:
                    nc.scalar.activation(vkeep[:, g * TG:g * TG + TG1, :],
                                         kgA[:, :, D:2 * D], AF.Copy)
                    if kgB is not None:
                        nc.scalar.activation(vkeep[:, g * TG + TG1:(g + 1) * TG, :],
                                             kgB[:, :, D:2 * D], AF.Copy)
                for t in range(TG):
                    kg_ap = kgA[:, t, 0:D] if t < TG1 else kgB[:, t - TG1, 0:D]
                    pq = bps.tile([128, D], dt.float32)
                    nc.tensor.matmul(pq[:], lhsT=stg_t[:, t, :], rhs=q_all[:, g, :],
                                     start=True, stop=True)
                    scr = scr_p.tile([128, D], dt.float16)
                    gt = g * TG + t
                    nc.vector.scalar_tensor_tensor(
                        out=scr[:], in0=pq[:], scalar=1.0, in1=kg_ap,
                        op0=OP.mult, op1=OP.mult,
                        accum_out=alpha_all[:, gt:gt + 1])

        # ================= stats + AllReduce + per-edge scale ======================
        with tc.tile_pool(name="stp", bufs=1) as stat_p, \
             tc.tile_pool(name="stps", bufs=2, space="PSUM") as stps:
            asq = stat_p.tile([128, G * TG], dt.float32)
            nc.vector.tensor_tensor(out=asq[:], in0=alpha_all[:], in1=alpha_all[:],
                                    op=OP.mult)
            st2 = stat_p.tile([128, 2], dt.float32)
            nc.vector.tensor_reduce(out=st2[:, 0:1], in_=alpha_all[:],
                                    axis=mybir.AxisListType.X, op=OP.add)
            nc.vector.tensor_reduce(out=st2[:, 1:2], in_=asq[:],
                                    axis=mybir.AxisListType.X, op=OP.add)
            ps1 = stps.tile([1, 2], dt.float32)
            nc.tensor.matmul(ps1[:], lhsT=ones_sb[:, 0:1], rhs=st2[:], start=True,
                             stop=True)
            ccs = stat_p.tile([1, 2], dt.float32)
            nc.vector.tensor_copy(ccs[:], ps1[:])
            nc.sync.dma_start(out=cc_in, in_=ccs[:])
            nc.gpsimd.collective_compute(
                "AllReduce", mybir.AluOpType.add, replica_groups=groups,
                ins=[cc_in], outs=[cc_out])
            ccr = stat_p.tile([1, 2], dt.float32)
            nc.sync.dma_start(out=ccr[:], in_=cc_out)
            # mu = S1/E ; var = (S2 - S1*mu)/(E-1) ; c = SCALE/sqrt(var)
            mu = stat_p.tile([1, 1], dt.float32)
            nc.vector.tensor_scalar(out=mu[:], in0=ccr[:, 0:1], scalar1=1.0 / E,
                                    scalar2=None, op0=OP.mult)
            var = stat_p.tile([1, 1], dt.float32)
            nc.vector.tensor_tensor(out=var[:], in0=ccr[:, 0:1], in1=mu[:], op=OP.mult)
            nc.vector.tensor_tensor(out=var[:], in0=ccr[:, 1:2], in1=var[:],
                                    op=OP.subtract)
            nc.vector.tensor_scalar(out=var[:], in0=var[:], scalar1=1.0 / (E - 1),
                                    scalar2=None, op0=OP.mult)
            nc.scalar.activation(var[:], var[:], AF.Sqrt)
            cfac = stat_p.tile([1, 1], dt.float32)
            nc.vector.reciprocal(cfac[:], var[:])
            nc.vector.tensor_scalar(out=cfac[:], in0=cfac[:],
                                    scalar1=float(SCALE_PARAM),
                                    scalar2=None, op0=OP.mult)
            mc = stat_p.tile([1, 2], dt.float32)
            nc.vector.tensor_copy(mc[:, 0:1], mu[:])
            nc.vector.tensor_copy(mc[:, 1:2], cfac[:])
            pb2 = stps.tile([128, 2], dt.float32)
            nc.tensor.matmul(pb2[:], lhsT=ones_sb[0:1, :], rhs=mc[:1, :], start=True,
                             stop=True)
            mc_col = stat_p.tile([128, 2], dt.float32)
            nc.vector.tensor_copy(mc_col[:], pb2[:])
            # scale = sigmoid((alpha - mu) * c)   (pad edges give garbage; killed by
            # the is_equal(seg=-1) indicator in sweep C)
            an = stat_p.tile([128, G * TG], dt.float32)
            nc.vector.tensor_scalar(out=an[:], in0=alpha_all[:],
                                    scalar1=mc_col[:, 0:1], scalar2=mc_col[:, 1:2],
                                    op0=OP.subtract, op1=OP.mult)
            nc.scalar.activation(an[:], an[:], AF.Sigmoid)
            scale16 = apool.tile([128, G * TG], dt.float16)
            nc.vector.tensor_copy(scale16[:], an[:])

        # ================= sweep C: output aggregation =============================
        # unscaled S streamed from host (e-part one-hot); per-edge scale folded
        # into v (per-partition scalar), split between Scalar and Vector engines.
        with tc.tile_pool(name="sc", bufs=3) as sc_p, \
             tc.tile_pool(name="vs", bufs=4) as vs_p, \
             tc.tile_pool(name="ops", bufs=2, space="PSUM") as ops, \
             tc.tile_pool(name="ot", bufs=2) as ot_p:
            for g in range(G):
                sg = sc_p.tile([128, TG, 128], dt.float16)
                nc.sync.dma_start(
                    out=sg[:],
                    in_=stc_in[:, g * TG * 128:(g + 1) * TG * 128].rearrange(
                        "p (t e) -> p t e", t=TG))
                vsc = vs_p.tile([128, TG, D], dt.float16)
                nc.vector.tensor_tensor(
                    out=vsc[:], in0=vkeep[:, g * TG:(g + 1) * TG, :],
                    in1=scale16[:, g * TG:(g + 1) * TG].unsqueeze(2)
                        .broadcast_to([128, TG, D]),
                    op=OP.mult)
                po = ops.tile([128, D], dt.float32)
                for t in range(TG):
                    nc.tensor.matmul(po[:], lhsT=sg[:, t, :], rhs=vsc[:, t, :],
                                     start=(t == 0), stop=False)
                # + root_weight skip via identity matmul
                nc.tensor.matmul(po[:], lhsT=identh_sb[:], rhs=s_all[:, g, :],
                                 start=False, stop=True)
                ot = ot_p.tile([128, D], dt.float32)
                if g % 2 == 0:
                    nc.vector.tensor_copy(ot[:], po[:])
                else:
                    nc.scalar.activation(ot[:], po[:], AF.Copy)
                nc.sync.dma_start(out=out_l[g * 128:(g + 1) * 128, :], in_=ot[:])

    nc.compile()
    return nc


# -------------------- driver -------------------------------------------------------

_CACHE = {}


def _get_program(cfg, TG, bias_zero, split_ok):
    key = (tuple(sorted((k, v) for k, v in cfg.items() if not isinstance(v, tuple))),
           cfg["GBK"], cfg["GBH"], TG, bias_zero, split_ok)
    if key not in _CACHE:
        _CACHE[key] = build_program(cfg, TG, bias_zero, split_ok)
    return _CACHE[key]


def run(inputs, cfg_base=None, trace=False):
    cfg = _derive(cfg_base or FULL_CFG)
    in_maps, dyn, unperm = prep_host(inputs, cfg)
    nc = _get_program(cfg, dyn["TG"], dyn["bias_zero"], dyn["split_ok"])
    from concourse.bass_utils import run_bass_kernel_spmd
    res = run_bass_kernel_spmd(nc, in_maps, list(range(cfg["C"])), trace=trace)
    full = np.concatenate([res.results[c]["out"] for c in range(cfg["C"])], 0)
    out = full[unperm["slot"]]
    return out.astype(np.float32), res


def kernel(**inputs):
    out, _ = run(inputs)
    return out


# revision 57
# speedup vs baseline: 1.0711x; 1.0711x over previous
"""GNN message-passing kernel (GCNConv + TransformerConv layer) for 8 Trainium2 cores.

V2 design (edges sharded by dst node; balanced host-side node permutation):
  * Host permutes nodes into 160 balanced groups of <=128 (snake-deal by in-degree)
    so every (core, group) has ~E/160 edges -> TG = ceil(max/128) = 16 tiles/group,
    and core shards are 2560-slot aligned (2500 real + pad), matching phase-1 blocks.
  * Phase 1: h0s = (x @ W_gcn) * dinv_src data-parallel over 512-node blocks (fp16);
    AllGather CHUNKED per block (5 AGs) so the exchange overlaps the matmuls.
    h0s rows of OWN nodes kept in SBUF (h0s_own) for the self-loop term.
  * Sweep A (GCN aggregation): per group dma_gather h0s[src] rows (no self loops),
    S-indicator built on DVE, segment-sum via PE matmul S^T @ G in PSUM; self-loop
    term added via identity matmul of h0s_own; copy-out = one scalar activation
    LeakyReLU(dinv_dst * x) (biases are zero -> exact fusion; general path kept).
  * Projections q,k,v,s per group (fp16); k|v packed -> kv_local; kv AllGather
    chunked over 5 group-ranges, interleaved with the aggregation loop.
  * Sweep B (alpha): gather kv[src] rows; QE = ST^T-matmul selects q[dst] per edge
    (ST = host-precomputed one-hot [s,e] streamed from DRAM); alpha = fused
    mult-mult-accum of QE * k on DVE. No per-tile transposes / copies.
  * Stats: tiny AllReduce of (sum, sumsq); sigmoid -> per-edge scale.
  * Sweep C: scaled S built per tile (DVE/GpSimd alternating), segment-sum of
    v[src] via PE; skip h@Ws added via identity matmul; out rows DMA'd per group.

Gathers ride 4 SWDGE queues round-robin; DMA drain (~33ns/512B desc/engine) is the
pacing resource, so everything else is spread across engines to hide behind it.
"""

from contextlib import ExitStack

import numpy as np

F16 = np.float16

# -------------------- problem constants (nn_DimEncoder_19894288515585) ------------
FULL_CFG = dict(N=20000, E=320000, F_IN=1024, H=256, D=128, C=8)
SCALE_PARAM = 3.0
LEAKY_SLOPE = 0.01


def _derive(cfg):
    N, C = cfg["N"], cfg["C"]
    d = dict(cfg)
    d["G"] = G = 20                      # groups (128-slot) per core
    d["NPC"] = NPC = G * 128             # 2560 slots per core
    d["NPAD"] = NPAD = C * NPC           # 20480 total slots
    assert NPAD >= N
    d["NBC"] = NBC = NPC // 512          # 512-row phase-1 blocks per core
    assert NBC * 512 == NPC
    d["KC"] = cfg["F_IN"] // 128
    d["HC"] = cfg["H"] // 128
    # kv AllGather chunk boundaries in groups (front-loaded, small tail).
    # Chunks 0-2 (groups < GBK[3]) form the "early" region: the first 8 tiles
    # of every group's edge list only reference srcs there, so their gathers
    # start before the tail chunks arrive.
    d["GBK"] = (0, 7, 12, 16, 19, G)
    # h0s AllGather chunks in 512-row blocks; blocks 0-2 (chunk 0) = early
    # region for the first-8-tile gathers.
    d["GBH"] = (0, 3, 5)
    return d


# -------------------- host-side preprocessing --------------------------------------

def _wrap_idx(a):
    """int16 [M] (M%16==0) -> dma_gather index layout [128, M//16]."""
    w = a.reshape(-1, 16).T.astype(np.int16)
    return np.tile(w, (8, 1))


def _balance_nodes(indeg, cfg):
    """Snake-deal nodes (sorted by in-degree desc) into C*G groups of <=128,
    then snake-deal groups into cores. Returns slot_of_node [N] (global pi-slot)."""
    N, C, G, NPC = cfg["N"], cfg["C"], cfg["G"], cfg["NPC"]
    NG = C * G
    order = np.argsort(-indeg, kind="stable")
    # group of the i-th node in sorted order: snake over NG bins
    i = np.arange(N)
    rnd, pos = i // NG, i % NG
    gbin = np.where(rnd % 2 == 0, pos, NG - 1 - pos)
    node_group = np.empty(N, np.int64)
    node_group[order] = gbin
    gload = np.bincount(node_group, weights=indeg, minlength=NG).astype(np.int64)
    # assign groups to cores: snake over sorted loads
    gorder = np.argsort(-gload, kind="stable")
    j = np.arange(NG)
    rndg, posg = j // C, j % C
    cbin = np.where(rndg % 2 == 0, posg, C - 1 - posg)
    group_core = np.empty(NG, np.int64)
    group_core[gorder] = cbin
    # local group index within core (order of appearance)
    gslot = np.full(NG, -1, np.int64)
    cnt = np.zeros(C, np.int64)
    for gid in range(NG):
        c = group_core[gid]
        gslot[gid] = cnt[c]
        cnt[c] += 1
    assert np.all(cnt == G)
    # slots within group: in node order
    slot_of_node = np.empty(N, np.int64)
    for gid in range(NG):
        nodes = np.where(node_group == gid)[0]
        assert len(nodes) <= 128
        c, gl = group_core[gid], gslot[gid]
        slot_of_node[nodes] = c * NPC + gl * 128 + np.arange(len(nodes))
    return slot_of_node


def prep_host(inputs, cfg):
    N, E, C = cfg["N"], cfg["E"], cfg["C"]
    NPC, G, NPAD, NBC = cfg["NPC"], cfg["G"], cfg["NPAD"], cfg["NBC"]
    KC, HC, F, H, D = cfg["KC"], cfg["HC"], cfg["F_IN"], cfg["H"], cfg["D"]
    GBK, GBH = cfg["GBK"], cfg["GBH"]

    x = np.asarray(inputs["x"], np.float32)
    ei = np.asarray(inputs["edge_index"])
    src, dst = ei[0].astype(np.int64), ei[1].astype(np.int64)

    indeg = np.bincount(dst, minlength=N)
    slot = _balance_nodes(indeg, cfg)

    deg = indeg + 1.0                         # + self loop
    dinv_node = 1.0 / np.sqrt(deg)
    dinv_slot = np.zeros(NPAD, np.float32)
    dinv_slot[slot] = dinv_node

    sd, ss = slot[dst], slot[src]
    # order edges by (dst group, src local-group): within each dst group, edges
    # whose src sits in the early AG regions come first -> their gather tiles
    # only touch the prefix of h0s_ext / kv_full.
    src_lg = (ss % NPC) // 128
    perm = np.lexsort((src_lg, sd // 128))
    sd, ss, src_lg = sd[perm], ss[perm], src_lg[perm]

    gcount = np.bincount(sd // 128, minlength=C * G)
    TG = max(1, int((gcount.max() + 127) // 128))
    L = TG * 128
    rp = np.zeros(C * G + 1, np.int64)
    rp[1:] = np.cumsum(gcount)

    # h0s_ext row for pi-slot p under the chunked (GBH block ranges) AG layout
    rbh = np.array(GBH) * 512
    rows_h = rbh[1:] - rbh[:-1]
    rowoff_h = np.concatenate([[0], np.cumsum(rows_h * C)])

    def h0row(p):
        pos = p % NPC
        j = np.searchsorted(rbh, pos, side="right") - 1
        return rowoff_h[j] + (p // NPC) * rows_h[j] + (pos - rbh[j])

    # kv_full row for pi-slot p under the chunked kv AllGather layout
    rb = np.array(GBK) * 128
    rows_j = rb[1:] - rb[:-1]
    rowoff = np.concatenate([[0], np.cumsum(rows_j * C)])

    def kvrow(p):
        pos = p % NPC
        j = np.searchsorted(rb, pos, side="right") - 1
        return rowoff[j] + (p // NPC) * rows_j[j] + (pos - rb[j])

    # pack edges per (core, group): seg (-1 pad), idxa (h0s row), idxkv (kv row)
    seg_p = np.full((C, G, L), -1, np.int64)
    ia_p = np.zeros((C, G, L), np.int64)
    ik_p = np.zeros((C, G, L), np.int64)
    split_ok = True
    for c in range(C):
        for g in range(G):
            gid = c * G + g
            i0, i1 = rp[gid], rp[gid + 1]
            n = i1 - i0
            seg_p[c, g, :n] = sd[i0:i1] - gid * 128
            ia_p[c, g, :n] = h0row(ss[i0:i1])
            ik_p[c, g, :n] = kvrow(ss[i0:i1])
            # first 8 tiles must only reference the early regions
            # (h0s chunk 0 = groups < GBH[1]*4; kv chunks 0-1 = groups < GBK[2])
            ncut = min(n, 8 * 128)
            lg = src_lg[i0:i0 + ncut]
            if len(lg) and (lg.max() >= GBH[1] * 4 or lg.max() >= GBK[2]):
                split_ok = False

    # ---- shared arrays
    xp = np.zeros((NPAD, F), np.float32)
    xp[slot] = x
    NB = NPAD // 512
    xt = np.ascontiguousarray(
        xp.reshape(NB, 512, KC, 128).transpose(0, 3, 2, 1)).astype(F16)

    wg = np.ascontiguousarray(
        np.asarray(inputs["W_gcn"], np.float32).reshape(KC, 128, H).transpose(1, 0, 2)
    ).astype(F16)

    def w2(name):
        w = np.asarray(inputs[name], np.float32).reshape(HC, 128, D).transpose(1, 0, 2)
        return np.ascontiguousarray(w).astype(F16)

    bias_zero = all(
        not np.any(np.asarray(inputs[b]))
        for b in ("b_gcn", "bq", "bk", "bv", "bs"))

    shared = {
        "wg": wg,
        "wq": w2("Wq"), "wk": w2("Wk"), "wv": w2("Wv"), "ws": w2("Ws"),
        "bg": np.asarray(inputs["b_gcn"], np.float32).reshape(1, H).astype(F16),
        "bq": np.asarray(inputs["bq"], np.float32).reshape(1, D).astype(F16),
        "bk": np.asarray(inputs["bk"], np.float32).reshape(1, D).astype(F16),
        "bv": np.asarray(inputs["bv"], np.float32).reshape(1, D).astype(F16),
        "bs": np.asarray(inputs["bs"], np.float32).reshape(1, D).astype(F16),
        "iota": np.tile(np.arange(128, dtype=np.float32)[None, :], (128, 1)).astype(F16),
        "identh": np.eye(128, dtype=F16),
        "ident": np.eye(128, dtype=np.float32),
        "ones": np.ones((128, 128), np.float32),
        "onesb": np.ones((1, 128), F16),
    }

    cols = np.arange(L)
    in_maps = []
    for c in range(C):
        m = dict(shared)
        m["xt"] = np.ascontiguousarray(xt[c * NBC:(c + 1) * NBC])
        m["dinv"] = dinv_slot[c * NPC:(c + 1) * NPC].reshape(G, 128).T.copy()
        m["idxa"] = np.concatenate([_wrap_idx(ia_p[c, g]) for g in range(G)], 1)
        m["idxkv"] = np.concatenate([_wrap_idx(ik_p[c, g]) for g in range(G)], 1)
        segc = seg_p[c].reshape(G, TG, 128).transpose(2, 0, 1).reshape(128, G * TG)
        m["sega"] = segc.astype(F16)
        m["seg32"] = segc.astype(np.float32)
        # ST one-hot [s, (g,t,e)] fp16 (sweep B: QE = ST^T @ q)
        st = np.zeros((128, G * TG * 128), F16)
        for g in range(G):
            sg = seg_p[c, g]
            valid = sg >= 0
            st[sg[valid], g * L + cols[valid]] = 1.0
        m["st"] = st
        # S one-hot e-part [e, (g,t,s)] fp16 (sweep C aggregation lhsT)
        stc = np.zeros((128, G * TG * 128), F16)
        e_in_tile = cols % 128
        tile_of = cols // 128
        for g in range(G):
            sg = seg_p[c, g]
            valid = sg >= 0
            stc[e_in_tile[valid], (g * TG + tile_of[valid]) * 128 + sg[valid]] = 1.0
        m["stc"] = stc
        in_maps.append(m)

    out_unperm = dict(slot=slot)
    return in_maps, dict(TG=TG, bias_zero=bias_zero, split_ok=split_ok), out_unperm


# -------------------- device program ----------------------------------------------

def build_program(cfg, TG, bias_zero, split_ok):
    import concourse.bacc as bacc
    import concourse.mybir as mybir
    from concourse.tile import TileContext

    dt = mybir.dt
    AF = mybir.ActivationFunctionType
    OP = mybir.AluOpType

    N, E, C = cfg["N"], cfg["E"], cfg["C"]
    NPC, G, NPAD, NBC = cfg["NPC"], cfg["G"], cfg["NPAD"], cfg["NBC"]
    KC, HC, H, D = cfg["KC"], cfg["HC"], cfg["H"], cfg["D"]
    GBK, GBH = cfg["GBK"], cfg["GBH"]
    _rb = [b * 128 for b in GBK]
    NCH = len(GBK) - 1
    _rowoff = [0]
    for j in range(NCH):
        _rowoff.append(_rowoff[-1] + (_rb[j + 1] - _rb[j]) * C)
    _rbh = [b * 512 for b in GBH]
    NCHH = len(GBH) - 1
    _rowoff_h = [0]
    for j in range(NCHH):
        _rowoff_h.append(_rowoff_h[-1] + (_rbh[j + 1] - _rbh[j]) * C)

    nc = bacc.Bacc("TRN2", target_bir_lowering=False, debug=False, num_devices=C,
                   num_swdge_queues=4)

    def din(name, shape, dtype):
        return nc.dram_tensor(name, list(shape), dtype, kind="ExternalInput").ap()

    xt = din("xt", [NBC, 128, KC, 512], dt.float16)
    wg = din("wg", [128, KC, H], dt.float16)
    wq, wk = din("wq", [128, HC, D], dt.float16), din("wk", [128, HC, D], dt.float16)
    wv, ws = din("wv", [128, HC, D], dt.float16), din("ws", [128, HC, D], dt.float16)
    bg = din("bg", [1, H], dt.float16)
    bq, bk = din("bq", [1, D], dt.float16), din("bk", [1, D], dt.float16)
    bv, bs = din("bv", [1, D], dt.float16), din("bs", [1, D], dt.float16)
    iota = din("iota", [128, 128], dt.float16)
    identh = din("identh", [128, 128], dt.float16)
    ident = din("ident", [128, 128], dt.float32)
    ones = din("ones", [128, 128], dt.float32)
    onesb = din("onesb", [1, 128], dt.float16)
    dinv = din("dinv", [128, G], dt.float32)
    idxa = din("idxa", [128, G * TG * 8], dt.int16)
    idxkv = din("idxkv", [128, G * TG * 8], dt.int16)
    sega = din("sega", [128, G * TG], dt.float16)
    seg32 = din("seg32", [128, G * TG], dt.float32)
    st_in = din("st", [128, G * TG * 128], dt.float16)
    stc_in = din("stc", [128, G * TG * 128], dt.float16)

    out_l = nc.dram_tensor("out", [NPC, D], dt.float32, kind="ExternalOutput").ap()

    h0s_loc = nc.dram_tensor("h0s_loc", [NPC, H], dt.float16).ap()
    h0s_ext = nc.dram_tensor("h0s_ext", [NPAD, H], dt.float16, addr_space="Shared").ap()
    kv_local = nc.dram_tensor("kv_local", [NPC, 2 * D], dt.float16).ap()
    kv_full = nc.dram_tensor("kv_full", [NPAD, 2 * D], dt.float16,
                             addr_space="Shared").ap()
    cc_in = nc.dram_tensor("cc_in", [1, 2], dt.float32).ap()
    cc_out = nc.dram_tensor("cc_out", [1, 2], dt.float32, addr_space="Shared").ap()

    groups = [list(range(C))]
    _gq = [0]

    def gather_tiles(out3, src_ap, idx_sb, g, t0, t1, elem):
        """Gather tiles [t0, t1) of group g into out3[:, 0:t1-t0, :]."""
        nc.gpsimd.dma_gather(
            out_ap=out3[:, 0:t1 - t0, :], in_ap=src_ap,
            idxs_ap=idx_sb[:, g * TG * 8 + t0 * 8:g * TG * 8 + t1 * 8],
            num_idxs=(t1 - t0) * 128, num_idxs_reg=(t1 - t0) * 128,
            elem_size=elem, queue_num=_gq[0])
        _gq[0] = (_gq[0] + 1) % 4

    with TileContext(nc) as tc, ExitStack() as ctx:
        cpool = ctx.enter_context(tc.tile_pool(name="consts", bufs=1))
        _cn = [0]

        def load_const(ap_in, shape, dtype):
            _cn[0] += 1
            t = cpool.tile(shape, dtype, tag=f"const{_cn[0]}")
            nc.sync.dma_start(out=t[:], in_=ap_in)
            return t

        wg_sb = load_const(wg, [128, KC, H], dt.float16)
        dinv_sb = load_const(dinv, [128, G], dt.float32)

        # persistent SBUF
        hpool = ctx.enter_context(tc.tile_pool(name="keep", bufs=1))
        h0s_own = hpool.tile([128, G, H], dt.float16)     # own h0s rows (self loops)
        q_all = hpool.tile([128, G, D], dt.float16)
        s_all = hpool.tile([128, G, D], dt.float16)
        apool = ctx.enter_context(tc.tile_pool(name="alpha", bufs=1))
        alpha_all = apool.tile([128, G * TG], dt.float32)
        vkeep = apool.tile([128, G * TG, D], dt.float16)

        # ================= phase 1: h0s node-block shard + chunked AllGather =======
        with tc.tile_pool(name="xt_p", bufs=2) as xt_p, \
             tc.tile_pool(name="h0ps", bufs=4, space="PSUM") as h0ps:
            for tb in range(NBC):
                xtile = xt_p.tile([128, KC, 512], dt.float16)
                nc.sync.dma_start(out=xtile[:], in_=xt[tb])
                for j in range(4):
                    g = tb * 4 + j
                    ph = h0ps.tile([128, H], dt.float32)
                    for k in range(KC):
                        nc.tensor.matmul(ph[:],
                                         lhsT=xtile[:, k, j * 128:(j + 1) * 128],
                                         rhs=wg_sb[:, k, :],
                                         start=(k == 0), stop=(k == KC - 1))
                    if j % 2 == 0:
                        nc.vector.tensor_scalar(out=h0s_own[:, g, :], in0=ph[:],
                                                scalar1=dinv_sb[:, g:g + 1],
                                                scalar2=None, op0=OP.mult)
                    else:
                        nc.scalar.activation(h0s_own[:, g, :], ph[:], AF.Copy,
                                             scale=dinv_sb[:, g:g + 1])
                nc.sync.dma_start(
                    out=h0s_loc[tb * 512:(tb + 1) * 512, :].rearrange(
                        "(j p) h -> p j h", p=128),
                    in_=h0s_own[:, tb * 4:(tb + 1) * 4, :])
                if (tb + 1) in GBH[1:]:
                    j = GBH[1:].index(tb + 1)
                    nc.gpsimd.collective_compute(
                        "AllGather", mybir.AluOpType.bypass, replica_groups=groups,
                        ins=[h0s_loc[_rbh[j]:_rbh[j + 1], :]],
                        outs=[h0s_ext[_rowoff_h[j]:_rowoff_h[j + 1], :]])

        # gather-phase constants (loaded after phase 1 so x tiles go first)
        iota_sb = load_const(iota, [128, 128], dt.float16)
        identh_sb = load_const(identh, [128, 128], dt.float16)
        ones_sb = load_const(ones, [128, 128], dt.float32)
        onesb_sb = load_const(onesb, [1, 128], dt.float16)
        w_sb = {n: load_const(a, [128, HC, D], dt.float16)
                for n, a in (("q", wq), ("k", wk), ("v", wv), ("s", ws))}
        idxa_sb = load_const(idxa, [128, G * TG * 8], dt.int16)
        idxkv_sb = load_const(idxkv, [128, G * TG * 8], dt.int16)
        sega_sb = load_const(sega, [128, G * TG], dt.float16)
        seg32_sb = load_const(seg32, [128, G * TG], dt.float32)

        b_sb = bgb_sb = None
        if not bias_zero:
            b_sb = {n: load_const(a, [1, D], dt.float16)
                    for n, a in (("q", bq), ("k", bk), ("v", bv), ("s", bs))}
            bg_sb = load_const(bg, [1, H], dt.float16)
            with tc.tile_pool(name="psb", bufs=1, space="PSUM") as psb:
                pb = psb.tile([128, H], dt.float32)
                nc.tensor.matmul(pb[:], lhsT=onesb_sb[:1, :], rhs=bg_sb[:1, :],
                                 start=True, stop=True)
                bgb_sb = cpool.tile([128, H], dt.float32)
                nc.vector.tensor_copy(bgb_sb[:], pb[:])

        # ============ sweep A: GCN aggregation + projections + kv exchange =========
        TG1 = min(8, TG)            # early tiles (srcs in h0s AG chunk 0)
        h0s_early = h0s_ext[0:_rowoff_h[1], :] if split_ok else h0s_ext
        with tc.tile_pool(name="gaA", bufs=11) as gaA_p, \
             tc.tile_pool(name="gaB", bufs=4) as gaB_p, \
             tc.tile_pool(name="sa", bufs=3) as sa_p, \
             tc.tile_pool(name="aps", bufs=2, space="PSUM") as aps, \
             tc.tile_pool(name="hsb", bufs=2) as hsb_p, \
             tc.tile_pool(name="ht", bufs=2) as ht_p, \
             tc.tile_pool(name="tps", bufs=2, space="PSUM") as tps, \
             tc.tile_pool(name="qps", bufs=4, space="PSUM") as qps, \
             tc.tile_pool(name="stg", bufs=2) as stg:
            gaA_t = []
            for g in range(G):
                gaA = gaA_p.tile([128, TG1, H], dt.float16)
                gather_tiles(gaA, h0s_early, idxa_sb, g, 0, TG1, H)
                gaA_t.append(gaA)
            for g in range(G):
                gaB = None
                if TG > TG1:
                    gaB = gaB_p.tile([128, TG - TG1, H], dt.float16)
                    gather_tiles(gaB, h0s_ext, idxa_sb, g, TG1, TG, H)
                gaA = gaA_t[g]
                sg = sa_p.tile([128, TG, 128], dt.float16)
                nc.vector.tensor_tensor(
                    out=sg[:],
                    in0=iota_sb[:].unsqueeze(1).broadcast_to([128, TG, 128]),
                    in1=sega_sb[:, g * TG:(g + 1) * TG].unsqueeze(2)
                        .broadcast_to([128, TG, 128]),
                    op=OP.is_equal)
                ph = aps.tile([128, H], dt.float32)
                for t in range(TG):
                    ga_ap = gaA[:, t, :] if t < TG1 else gaB[:, t - TG1, :]
                    nc.tensor.matmul(ph[:], lhsT=sg[:, t, :], rhs=ga_ap,
                                     start=(t == 0), stop=False)
                # + self-loop term via identity matmul of own h0s rows
                nc.tensor.matmul(ph[:], lhsT=identh_sb[:], rhs=h0s_own[:, g, :],
                                 start=False, stop=True)
                h16 = hsb_p.tile([128, H], dt.float16)
                if bias_zero:
                    # LeakyReLU(dinv*x) == dinv*LeakyReLU(x), dinv > 0
                    nc.scalar.activation(h16[:], ph[:], AF.Lrelu,
                                         scale=dinv_sb[:, g:g + 1],
                                         alpha=LEAKY_SLOPE)
                else:
                    hf = hsb_p.tile([128, H], dt.float32, tag="hf")
                    nc.vector.tensor_scalar(out=hf[:], in0=ph[:],
                                            scalar1=dinv_sb[:, g:g + 1],
                                            scalar2=None, op0=OP.mult)
                    nc.vector.tensor_tensor(out=hf[:], in0=hf[:], in1=bgb_sb[:],
                                            op=OP.add)
                    nc.scalar.activation(h16[:], hf[:], AF.Lrelu, alpha=LEAKY_SLOPE)
                # ---- layer-2 projections for this group
                ht = ht_p.tile([128, HC, 128], dt.float16)
                for hc in range(HC):
                    pt = tps.tile([128, 128], dt.float16)
                    nc.tensor.transpose(pt[:], h16[:, hc * 128:(hc + 1) * 128],
                                        identh_sb[:])
                    eng = (nc.vector, nc.scalar)[hc % 2]
                    if hc % 2 == 0:
                        nc.vector.tensor_copy(ht[:, hc, :], pt[:])
                    else:
                        nc.scalar.activation(ht[:, hc, :], pt[:], AF.Copy)
                kv_st = stg.tile([128, 2, D], dt.float16, tag="kv_st")
                for i, name in enumerate(("q", "k", "v", "s")):
                    pq = qps.tile([128, D], dt.float32)
                    for hc in range(HC):
                        last = (hc == HC - 1) and bias_zero
                        nc.tensor.matmul(pq[:], lhsT=ht[:, hc, :],
                                         rhs=w_sb[name][:, hc, :],
                                         start=(hc == 0), stop=last)
                    if not bias_zero:
                        nc.tensor.matmul(pq[:], lhsT=onesb_sb[:1, :],
                                         rhs=b_sb[name][:1, :],
                                         start=False, stop=True)
                    dst_ap = {"q": q_all[:, g, :], "k": kv_st[:, 0, :],
                              "v": kv_st[:, 1, :], "s": s_all[:, g, :]}[name]
                    if i % 2 == 0:
                        nc.vector.tensor_copy(dst_ap, pq[:])
                    else:
                        nc.scalar.activation(dst_ap, pq[:], AF.Copy)
                nc.sync.dma_start(out=kv_local[g * 128:(g + 1) * 128, :],
                                  in_=kv_st[:].rearrange("p a b -> p (a b)"))
                if (g + 1) in GBK[1:]:
                    j = GBK[1:].index(g + 1)
                    nc.gpsimd.collective_compute(
                        "AllGather", mybir.AluOpType.bypass, replica_groups=groups,
                        ins=[kv_local[_rb[j]:_rb[j + 1], :]],
                        outs=[kv_full[_rowoff[j]:_rowoff[j + 1], :]])

        # ================= sweep B: alpha ==========================================
        kv_early = kv_full[0:_rowoff[2], :] if split_ok else kv_full
        with tc.tile_pool(name="kgA", bufs=11) as kgA_p, \
             tc.tile_pool(name="kgB", bufs=4) as kgB_p, \
             tc.tile_pool(name="stt", bufs=3) as st_p, \
             tc.tile_pool(name="bps", bufs=4, space="PSUM") as bps, \
             tc.tile_pool(name="scr", bufs=3) as scr_p:
            kgA_t = []
            for g in range(G):
                kgA = kgA_p.tile([128, TG1, 2 * D], dt.float16)
                gather_tiles(kgA, kv_early, idxkv_sb, g, 0, TG1, 2 * D)
                kgA_t.append(kgA)
            for g in range(G):
                kgB = None
                if TG > TG1:
                    kgB = kgB_p.tile([128, TG - TG1, 2 * D], dt.float16)
                    gather_tiles(kgB, kv_full, idxkv_sb, g, TG1, TG, 2 * D)
                kgA = kgA_t[g]
                stg_t = st_p.tile([128, TG, 128], dt.float16)
                nc.sync.dma_start(
                    out=stg_t[:],
                    in_=st_in[:, g * TG * 128:(g + 1) * TG * 128].rearrange(
                        "p (t e) -> p t e", t=TG))
                if g % 2 == 0:
                    nc.vector.tensor_copy(vkeep[:, g * TG:g * TG + TG1, :],
                                          kgA[:, :, D:2 * D])
                    if kgB is not None:
                        nc.vector.tensor_copy(vkeep[:, g * TG + TG1:(g + 1) * TG, :],
                                              kgB[:, :, D:2 * D])
                else:
                    nc.scalar.activation(vkeep[:, g * TG:g * TG + TG1, :],
                                         kgA[:, :, D:2 * D], AF.Copy)
                    if kgB is not None:
                        nc.scalar.activation(vkeep[:, g * TG + TG1:(g + 1) * TG, :],
                                             kgB[:, :, D:2 * D], AF.Copy)
                for t in range(TG):
                    kg_ap = kgA[:, t, 0:D] if t < TG1 else kgB[:, t - TG1, 0:D]
                    pq = bps.tile([128, D], dt.float32)
                    nc.tensor.matmul(pq[:], lhsT=stg_t[:, t, :], rhs=q_all[:, g, :],
                                     start=True, stop=True)
                    scr = scr_p.tile([128, D], dt.float16)
                    gt = g * TG + t
                    nc.vector.scalar_tensor_tensor(
                        out=scr[:], in0=pq[:], scalar=1.0, in1=kg_ap,
                        op0=OP.mult, op1=OP.mult,
                        accum_out=alpha_all[:, gt:gt + 1])

        # ================= stats + AllReduce + per-edge scale ======================
        with tc.tile_pool(name="stp", bufs=1) as stat_p, \
             tc.tile_pool(name="stps", bufs=2, space="PSUM") as stps:
            asq = stat_p.tile([128, G * TG], dt.float32)
            nc.vector.tensor_tensor(out=asq[:], in0=alpha_all[:], in1=alpha_all[:],
                                    op=OP.mult)
            st2 = stat_p.tile([128, 2], dt.float32)
            nc.vector.tensor_reduce(out=st2[:, 0:1], in_=alpha_all[:],
                                    axis=mybir.AxisListType.X, op=OP.add)
            nc.vector.tensor_reduce(out=st2[:, 1:2], in_=asq[:],
                                    axis=mybir.AxisListType.X, op=OP.add)
            ps1 = stps.tile([1, 2], dt.float32)
            nc.tensor.matmul(ps1[:], lhsT=ones_sb[:, 0:1], rhs=st2[:], start=True,
                             stop=True)
            ccs = stat_p.tile([1, 2], dt.float32)
            nc.vector.tensor_copy(ccs[:], ps1[:])
            nc.sync.dma_start(out=cc_in, in_=ccs[:])
            nc.gpsimd.collective_compute(
                "AllReduce", mybir.AluOpType.add, replica_groups=groups,
                ins=[cc_in], outs=[cc_out])
            ccr = stat_p.tile([1, 2], dt.float32)
            nc.sync.dma_start(out=ccr[:], in_=cc_out)
            # mu = S1/E ; var = (S2 - S1*mu)/(E-1) ; c = SCALE/sqrt(var)
            mu = stat_p.tile([1, 1], dt.float32)
            nc.vector.tensor_scalar(out=mu[:], in0=ccr[:, 0:1], scalar1=1.0 / E,
                                    scalar2=None, op0=OP.mult)
            var = stat_p.tile([1, 1], dt.float32)
            nc.vector.tensor_tensor(out=var[:], in0=ccr[:, 0:1], in1=mu[:], op=OP.mult)
            nc.vector.tensor_tensor(out=var[:], in0=ccr[:, 1:2], in1=var[:],
                                    op=OP.subtract)
            nc.vector.tensor_scalar(out=var[:], in0=var[:], scalar1=1.0 / (E - 1),
                                    scalar2=None, op0=OP.mult)
            nc.scalar.activation(var[:], var[:], AF.Sqrt)
            cfac = stat_p.tile([1, 1], dt.float32)
            nc.vector.reciprocal(cfac[:], var[:])
            nc.vector.tensor_scalar(out=cfac[:], in0=cfac[:],
                                    scalar1=float(SCALE_PARAM),
                                    scalar2=None, op0=OP.mult)
            mc = stat_p.tile([1, 2], dt.float32)
            nc.vector.tensor_copy(mc[:, 0:1], mu[:])
            nc.vector.tensor_copy(mc[:, 1:2], cfac[:])
            pb2 = stps.tile([128, 2], dt.float32)
            nc.tensor.matmul(pb2[:], lhsT=ones_sb[0:1, :], rhs=mc[:1, :], start=True,
                             stop=True)
            mc_col = stat_p.tile([128, 2], dt.float32)
            nc.vector.tensor_copy(mc_col[:], pb2[:])
            # scale = sigmoid((alpha - mu) * c)   (pad edges give garbage; killed by
            # the is_equal(seg=-1) indicator in sweep C)
            an = stat_p.tile([128, G * TG], dt.float32)
            nc.vector.tensor_scalar(out=an[:], in0=alpha_all[:],
                                    scalar1=mc_col[:, 0:1], scalar2=mc_col[:, 1:2],
                                    op0=OP.subtract, op1=OP.mult)
            nc.scalar.activation(an[:], an[:], AF.Sigmoid)
            scale16 = apool.tile([128, G * TG], dt.float16)
            nc.vector.tensor_copy(scale16[:], an[:])

        # ================= sweep C: output aggregation =============================
        # unscaled S streamed from host (e-part one-hot); per-edge scale folded
        # into v (per-partition scalar), split between Scalar and Vector engines.
        with tc.tile_pool(name="sc", bufs=3) as sc_p, \
             tc.tile_pool(name="vs", bufs=4) as vs_p, \
             tc.tile_pool(name="ops", bufs=2, space="PSUM") as ops, \
             tc.tile_pool(name="ot", bufs=2) as ot_p:
            for g in range(G):
                sg = sc_p.tile([128, TG, 128], dt.float16)
                nc.sync.dma_start(
                    out=sg[:],
                    in_=stc_in[:, g * TG * 128:(g + 1) * TG * 128].rearrange(
                        "p (t e) -> p t e", t=TG))
                vsc = vs_p.tile([128, TG, D], dt.float16)
                nc.vector.tensor_tensor(
                    out=vsc[:], in0=vkeep[:, g * TG:(g + 1) * TG, :],
                    in1=scale16[:, g * TG:(g + 1) * TG].unsqueeze(2)
                        .broadcast_to([128, TG, D]),
                    op=OP.mult)
                po = ops.tile([128, D], dt.float32)
                for t in range(TG):
                    nc.tensor.matmul(po[:], lhsT=sg[:, t, :], rhs=vsc[:, t, :],
                                     start=(t == 0), stop=False)
                # + root_weight skip via identity matmul
                nc.tensor.matmul(po[:], lhsT=identh_sb[:], rhs=s_all[:, g, :],
                                 start=False, stop=True)
                ot = ot_p.tile([128, D], dt.float32)
                if g % 2 == 0:
                    nc.vector.tensor_copy(ot[:], po[:])
                else:
                    nc.scalar.activation(ot[:], po[:], AF.Copy)
                nc.sync.dma_start(out=out_l[g * 128:(g + 1) * 128, :], in_=ot[:])

    nc.compile()
    return nc


# -------------------- driver -------------------------------------------------------

_CACHE = {}


def _get_program(cfg, TG, bias_zero, split_ok):
    key = (tuple(sorted((k, v) for k, v in cfg.items() if not isinstance(v, tuple))),
           cfg["GBK"], cfg["GBH"], TG, bias_zero, split_ok)
    if key not in _CACHE:
        _CACHE[key] = build_program(cfg, TG, bias_zero, split_ok)
    return _CACHE[key]


def run(inputs, cfg_base=None, trace=False):
    cfg = _derive(cfg_base or FULL_CFG)
    in_maps, dyn, unperm = prep_host(inputs, cfg)
    nc = _get_program(cfg, dyn["TG"], dyn["bias_zero"], dyn["split_ok"])
    from concourse.bass_utils import run_bass_kernel_spmd
    res = run_bass_kernel_spmd(nc, in_maps, list(range(cfg["C"])), trace=trace)
    full = np.concatenate([res.results[c]["out"] for c in range(cfg["C"])], 0)
    out = full[unperm["slot"]]
    return out.astype(np.float32), res


def kernel(**inputs):
    out, _ = run(inputs)
    return out


# revision 58
# speedup vs baseline: 1.0820x; 1.0101x over previous
"""GNN message-passing kernel (GCNConv + TransformerConv layer) for 8 Trainium2 cores.

V2 design (edges sharded by dst node; balanced host-side node permutation):
  * Host permutes nodes into 160 balanced groups of <=128 (snake-deal by in-degree)
    so every (core, group) has ~E/160 edges -> TG = ceil(max/128) = 16 tiles/group,
    and core shards are 2560-slot aligned (2500 real + pad), matching phase-1 blocks.
  * Phase 1: h0s = (x @ W_gcn) * dinv_src data-parallel over 512-node blocks (fp16);
    AllGather CHUNKED per block (5 AGs) so the exchange overlaps the matmuls.
    h0s rows of OWN nodes kept in SBUF (h0s_own) for the self-loop term.
  * Sweep A (GCN aggregation): per group dma_gather h0s[src] rows (no self loops),
    S-indicator built on DVE, segment-sum via PE matmul S^T @ G in PSUM; self-loop
    term added via identity matmul of h0s_own; copy-out = one scalar activation
    LeakyReLU(dinv_dst * x) (biases are zero -> exact fusion; general path kept).
  * Projections q,k,v,s per group (fp16); k|v packed -> kv_local; kv AllGather
    chunked over 5 group-ranges, interleaved with the aggregation loop.
  * Sweep B (alpha): gather kv[src] rows; QE = ST^T-matmul selects q[dst] per edge
    (ST = host-precomputed one-hot [s,e] streamed from DRAM); alpha = fused
    mult-mult-accum of QE * k on DVE. No per-tile transposes / copies.
  * Stats: tiny AllReduce of (sum, sumsq); sigmoid -> per-edge scale.
  * Sweep C: scaled S built per tile (DVE/GpSimd alternating), segment-sum of
    v[src] via PE; skip h@Ws added via identity matmul; out rows DMA'd per group.

Gathers ride 4 SWDGE queues round-robin; DMA drain (~33ns/512B desc/engine) is the
pacing resource, so everything else is spread across engines to hide behind it.
"""

from contextlib import ExitStack

import numpy as np

F16 = np.float16

# -------------------- problem constants (nn_DimEncoder_19894288515585) ------------
FULL_CFG = dict(N=20000, E=320000, F_IN=1024, H=256, D=128, C=8)
SCALE_PARAM = 3.0
LEAKY_SLOPE = 0.01


def _derive(cfg):
    N, C = cfg["N"], cfg["C"]
    d = dict(cfg)
    d["G"] = G = 20                      # groups (128-slot) per core
    d["NPC"] = NPC = G * 128             # 2560 slots per core
    d["NPAD"] = NPAD = C * NPC           # 20480 total slots
    assert NPAD >= N
    d["NBC"] = NBC = NPC // 512          # 512-row phase-1 blocks per core
    assert NBC * 512 == NPC
    d["KC"] = cfg["F_IN"] // 128
    d["HC"] = cfg["H"] // 128
    # kv AllGather chunk boundaries in groups (front-loaded, small tail).
    # Chunks 0-2 (groups < GBK[3]) form the "early" region: the first 8 tiles
    # of every group's edge list only reference srcs there, so their gathers
    # start before the tail chunks arrive.
    d["GBK"] = (0, 7, 12, 16, 19, G)
    # h0s AllGather chunks in 512-row blocks; blocks 0-2 (chunk 0) = early
    # region for the first-8-tile gathers.
    d["GBH"] = (0, 3, 5)
    return d


# -------------------- host-side preprocessing --------------------------------------

def _wrap_idx(a):
    """int16 [M] (M%16==0) -> dma_gather index layout [128, M//16]."""
    w = a.reshape(-1, 16).T.astype(np.int16)
    return np.tile(w, (8, 1))


def _balance_nodes(indeg, cfg):
    """Snake-deal nodes (sorted by in-degree desc) into C*G groups of <=128,
    then snake-deal groups into cores. Returns slot_of_node [N] (global pi-slot)."""
    N, C, G, NPC = cfg["N"], cfg["C"], cfg["G"], cfg["NPC"]
    NG = C * G
    order = np.argsort(-indeg, kind="stable")
    # group of the i-th node in sorted order: snake over NG bins
    i = np.arange(N)
    rnd, pos = i // NG, i % NG
    gbin = np.where(rnd % 2 == 0, pos, NG - 1 - pos)
    node_group = np.empty(N, np.int64)
    node_group[order] = gbin
    gload = np.bincount(node_group, weights=indeg, minlength=NG).astype(np.int64)
    # assign groups to cores: snake over sorted loads
    gorder = np.argsort(-gload, kind="stable")
    j = np.arange(NG)
    rndg, posg = j // C, j % C
    cbin = np.where(rndg % 2 == 0, posg, C - 1 - posg)
    group_core = np.empty(NG, np.int64)
    group_core[gorder] = cbin
    # local group index within core (order of appearance)
    gslot = np.full(NG, -1, np.int64)
    cnt = np.zeros(C, np.int64)
    for gid in range(NG):
        c = group_core[gid]
        gslot[gid] = cnt[c]
        cnt[c] += 1
    assert np.all(cnt == G)
    # slots within group: in node order
    slot_of_node = np.empty(N, np.int64)
    for gid in range(NG):
        nodes = np.where(node_group == gid)[0]
        assert len(nodes) <= 128
        c, gl = group_core[gid], gslot[gid]
        slot_of_node[nodes] = c * NPC + gl * 128 + np.arange(len(nodes))
    return slot_of_node


def prep_host(inputs, cfg):
    N, E, C = cfg["N"], cfg["E"], cfg["C"]
    NPC, G, NPAD, NBC = cfg["NPC"], cfg["G"], cfg["NPAD"], cfg["NBC"]
    KC, HC, F, H, D = cfg["KC"], cfg["HC"], cfg["F_IN"], cfg["H"], cfg["D"]
    GBK, GBH = cfg["GBK"], cfg["GBH"]

    x = np.asarray(inputs["x"], np.float32)
    ei = np.asarray(inputs["edge_index"])
    src, dst = ei[0].astype(np.int64), ei[1].astype(np.int64)

    indeg = np.bincount(dst, minlength=N)
    slot = _balance_nodes(indeg, cfg)

    deg = indeg + 1.0                         # + self loop
    dinv_node = 1.0 / np.sqrt(deg)
    dinv_slot = np.zeros(NPAD, np.float32)
    dinv_slot[slot] = dinv_node

    sd, ss = slot[dst], slot[src]
    # order edges by (dst group, src local-group): within each dst group, edges
    # whose src sits in the early AG regions come first -> their gather tiles
    # only touch the prefix of h0s_ext / kv_full.
    src_lg = (ss % NPC) // 128
    perm = np.lexsort((src_lg, sd // 128))
    sd, ss, src_lg = sd[perm], ss[perm], src_lg[perm]

    gcount = np.bincount(sd // 128, minlength=C * G)
    TG = max(1, int((gcount.max() + 127) // 128))
    L = TG * 128
    rp = np.zeros(C * G + 1, np.int64)
    rp[1:] = np.cumsum(gcount)

    # h0s_ext row for pi-slot p under the chunked (GBH block ranges) AG layout
    rbh = np.array(GBH) * 512
    rows_h = rbh[1:] - rbh[:-1]
    rowoff_h = np.concatenate([[0], np.cumsum(rows_h * C)])

    def h0row(p):
        pos = p % NPC
        j = np.searchsorted(rbh, pos, side="right") - 1
        return rowoff_h[j] + (p // NPC) * rows_h[j] + (pos - rbh[j])

    # kv_full row for pi-slot p under the chunked kv AllGather layout
    rb = np.array(GBK) * 128
    rows_j = rb[1:] - rb[:-1]
    rowoff = np.concatenate([[0], np.cumsum(rows_j * C)])

    def kvrow(p):
        pos = p % NPC
        j = np.searchsorted(rb, pos, side="right") - 1
        return rowoff[j] + (p // NPC) * rows_j[j] + (pos - rb[j])

    # pack edges per (core, group): seg (-1 pad), idxa (h0s row), idxkv (kv row)
    seg_p = np.full((C, G, L), -1, np.int64)
    ia_p = np.zeros((C, G, L), np.int64)
    ik_p = np.zeros((C, G, L), np.int64)
    split_ok = True
    for c in range(C):
        for g in range(G):
            gid = c * G + g
            i0, i1 = rp[gid], rp[gid + 1]
            n = i1 - i0
            seg_p[c, g, :n] = sd[i0:i1] - gid * 128
            ia_p[c, g, :n] = h0row(ss[i0:i1])
            ik_p[c, g, :n] = kvrow(ss[i0:i1])
            # first 8 tiles must only reference the early regions
            # (h0s chunk 0 = groups < GBH[1]*4; kv chunks 0-1 = groups < GBK[2])
            ncut = min(n, 8 * 128)
            lg = src_lg[i0:i0 + ncut]
            if len(lg) and (lg.max() >= GBH[1] * 4 or lg.max() >= GBK[2]):
                split_ok = False

    # ---- shared arrays
    xp = np.zeros((NPAD, F), np.float32)
    xp[slot] = x
    NB = NPAD // 512
    xt = np.ascontiguousarray(
        xp.reshape(NB, 512, KC, 128).transpose(0, 3, 2, 1)).astype(F16)

    wg = np.ascontiguousarray(
        np.asarray(inputs["W_gcn"], np.float32).reshape(KC, 128, H).transpose(1, 0, 2)
    ).astype(F16)

    def w2(name):
        w = np.asarray(inputs[name], np.float32).reshape(HC, 128, D).transpose(1, 0, 2)
        return np.ascontiguousarray(w).astype(F16)

    bias_zero = all(
        not np.any(np.asarray(inputs[b]))
        for b in ("b_gcn", "bq", "bk", "bv", "bs"))

    shared = {
        "wg": wg,
        "wq": w2("Wq"), "wk": w2("Wk"), "wv": w2("Wv"), "ws": w2("Ws"),
        "bg": np.asarray(inputs["b_gcn"], np.float32).reshape(1, H).astype(F16),
        "bq": np.asarray(inputs["bq"], np.float32).reshape(1, D).astype(F16),
        "bk": np.asarray(inputs["bk"], np.float32).reshape(1, D).astype(F16),
        "bv": np.asarray(inputs["bv"], np.float32).reshape(1, D).astype(F16),
        "bs": np.asarray(inputs["bs"], np.float32).reshape(1, D).astype(F16),
        "iota": np.tile(np.arange(128, dtype=np.float32)[None, :], (128, 1)).astype(F16),
        "identh": np.eye(128, dtype=F16),
        "ident": np.eye(128, dtype=np.float32),
        "ones": np.ones((128, 128), np.float32),
        "onesb": np.ones((1, 128), F16),
    }

    cols = np.arange(L)
    in_maps = []
    for c in range(C):
        m = dict(shared)
        m["xt"] = np.ascontiguousarray(xt[c * NBC:(c + 1) * NBC])
        m["dinv"] = dinv_slot[c * NPC:(c + 1) * NPC].reshape(G, 128).T.copy()
        m["idxa"] = np.concatenate([_wrap_idx(ia_p[c, g]) for g in range(G)], 1)
        m["idxkv"] = np.concatenate([_wrap_idx(ik_p[c, g]) for g in range(G)], 1)
        segc = seg_p[c].reshape(G, TG, 128).transpose(2, 0, 1).reshape(128, G * TG)
        m["sega"] = segc.astype(F16)
        m["seg32"] = segc.astype(np.float32)
        # ST one-hot [s, (g,t,e)] fp16 (sweep B: QE = ST^T @ q)
        st = np.zeros((128, G * TG * 128), F16)
        for g in range(G):
            sg = seg_p[c, g]
            valid = sg >= 0
            st[sg[valid], g * L + cols[valid]] = 1.0
        m["st"] = st
        # S one-hot e-part [e, (g,t,s)] fp16 (sweep C aggregation lhsT)
        stc = np.zeros((128, G * TG * 128), F16)
        e_in_tile = cols % 128
        tile_of = cols // 128
        for g in range(G):
            sg = seg_p[c, g]
            valid = sg >= 0
            stc[e_in_tile[valid], (g * TG + tile_of[valid]) * 128 + sg[valid]] = 1.0
        m["stc"] = stc
        in_maps.append(m)

    out_unperm = dict(slot=slot)
    return in_maps, dict(TG=TG, bias_zero=bias_zero, split_ok=split_ok), out_unperm


# -------------------- device program ----------------------------------------------

def build_program(cfg, TG, bias_zero, split_ok):
    import concourse.bacc as bacc
    import concourse.mybir as mybir
    from concourse.tile import TileContext

    dt = mybir.dt
    AF = mybir.ActivationFunctionType
    OP = mybir.AluOpType

    N, E, C = cfg["N"], cfg["E"], cfg["C"]
    NPC, G, NPAD, NBC = cfg["NPC"], cfg["G"], cfg["NPAD"], cfg["NBC"]
    KC, HC, H, D = cfg["KC"], cfg["HC"], cfg["H"], cfg["D"]
    GBK, GBH = cfg["GBK"], cfg["GBH"]
    _rb = [b * 128 for b in GBK]
    NCH = len(GBK) - 1
    _rowoff = [0]
    for j in range(NCH):
        _rowoff.append(_rowoff[-1] + (_rb[j + 1] - _rb[j]) * C)
    _rbh = [b * 512 for b in GBH]
    NCHH = len(GBH) - 1
    _rowoff_h = [0]
    for j in range(NCHH):
        _rowoff_h.append(_rowoff_h[-1] + (_rbh[j + 1] - _rbh[j]) * C)

    nc = bacc.Bacc("TRN2", target_bir_lowering=False, debug=False, num_devices=C,
                   num_swdge_queues=4)

    def din(name, shape, dtype):
        return nc.dram_tensor(name, list(shape), dtype, kind="ExternalInput").ap()

    xt = din("xt", [NBC, 128, KC, 512], dt.float16)
    wg = din("wg", [128, KC, H], dt.float16)
    wq, wk = din("wq", [128, HC, D], dt.float16), din("wk", [128, HC, D], dt.float16)
    wv, ws = din("wv", [128, HC, D], dt.float16), din("ws", [128, HC, D], dt.float16)
    bg = din("bg", [1, H], dt.float16)
    bq, bk = din("bq", [1, D], dt.float16), din("bk", [1, D], dt.float16)
    bv, bs = din("bv", [1, D], dt.float16), din("bs", [1, D], dt.float16)
    iota = din("iota", [128, 128], dt.float16)
    identh = din("identh", [128, 128], dt.float16)
    ident = din("ident", [128, 128], dt.float32)
    ones = din("ones", [128, 128], dt.float32)
    onesb = din("onesb", [1, 128], dt.float16)
    dinv = din("dinv", [128, G], dt.float32)
    idxa = din("idxa", [128, G * TG * 8], dt.int16)
    idxkv = din("idxkv", [128, G * TG * 8], dt.int16)
    sega = din("sega", [128, G * TG], dt.float16)
    seg32 = din("seg32", [128, G * TG], dt.float32)
    st_in = din("st", [128, G * TG * 128], dt.float16)
    stc_in = din("stc", [128, G * TG * 128], dt.float16)

    out_l = nc.dram_tensor("out", [NPC, D], dt.float32, kind="ExternalOutput").ap()

    h0s_loc = nc.dram_tensor("h0s_loc", [NPC, H], dt.float16).ap()
    h0s_ext = nc.dram_tensor("h0s_ext", [NPAD, H], dt.float16, addr_space="Shared").ap()
    kv_local = nc.dram_tensor("kv_local", [NPC, 2 * D], dt.float16).ap()
    kv_full = nc.dram_tensor("kv_full", [NPAD, 2 * D], dt.float16,
                             addr_space="Shared").ap()
    cc_in = nc.dram_tensor("cc_in", [1, 2], dt.float32).ap()
    cc_out = nc.dram_tensor("cc_out", [1, 2], dt.float32, addr_space="Shared").ap()

    groups = [list(range(C))]
    _gq = [0]

    def gather_tiles(out3, src_ap, idx_sb, g, t0, t1, elem):
        """Gather tiles [t0, t1) of group g into out3[:, 0:t1-t0, :]."""
        nc.gpsimd.dma_gather(
            out_ap=out3[:, 0:t1 - t0, :], in_ap=src_ap,
            idxs_ap=idx_sb[:, g * TG * 8 + t0 * 8:g * TG * 8 + t1 * 8],
            num_idxs=(t1 - t0) * 128, num_idxs_reg=(t1 - t0) * 128,
            elem_size=elem, queue_num=_gq[0])
        _gq[0] = (_gq[0] + 1) % 4

    with TileContext(nc) as tc, ExitStack() as ctx:
        cpool = ctx.enter_context(tc.tile_pool(name="consts", bufs=1))
        _cn = [0]

        def load_const(ap_in, shape, dtype):
            _cn[0] += 1
            t = cpool.tile(shape, dtype, tag=f"const{_cn[0]}")
            nc.sync.dma_start(out=t[:], in_=ap_in)
            return t

        wg_sb = load_const(wg, [128, KC, H], dt.float16)
        dinv_sb = load_const(dinv, [128, G], dt.float32)

        # persistent SBUF
        hpool = ctx.enter_context(tc.tile_pool(name="keep", bufs=1))
        h0s_own = hpool.tile([128, G, H], dt.float16)     # own h0s rows (self loops)
        q_all = hpool.tile([128, G, D], dt.float16)
        s_all = hpool.tile([128, G, D], dt.float16)
        apool = ctx.enter_context(tc.tile_pool(name="alpha", bufs=1))
        alpha_all = apool.tile([128, G * TG], dt.float32)
        vkeep = apool.tile([128, G * TG, D], dt.float16)

        # ================= phase 1: h0s node-block shard + chunked AllGather =======
        with tc.tile_pool(name="xt_p", bufs=2) as xt_p, \
             tc.tile_pool(name="h0ps", bufs=4, space="PSUM") as h0ps:
            for tb in range(NBC):
                xtile = xt_p.tile([128, KC, 512], dt.float16)
                nc.sync.dma_start(out=xtile[:], in_=xt[tb])
                for j in range(4):
                    g = tb * 4 + j
                    ph = h0ps.tile([128, H], dt.float32)
                    for k in range(KC):
                        nc.tensor.matmul(ph[:],
                                         lhsT=xtile[:, k, j * 128:(j + 1) * 128],
                                         rhs=wg_sb[:, k, :],
                                         start=(k == 0), stop=(k == KC - 1))
                    if j % 2 == 0:
                        nc.vector.tensor_scalar(out=h0s_own[:, g, :], in0=ph[:],
                                                scalar1=dinv_sb[:, g:g + 1],
                                                scalar2=None, op0=OP.mult)
                    else:
                        nc.scalar.activation(h0s_own[:, g, :], ph[:], AF.Copy,
                                             scale=dinv_sb[:, g:g + 1])
                nc.sync.dma_start(
                    out=h0s_loc[tb * 512:(tb + 1) * 512, :].rearrange(
                        "(j p) h -> p j h", p=128),
                    in_=h0s_own[:, tb * 4:(tb + 1) * 4, :])
                if (tb + 1) in GBH[1:]:
                    j = GBH[1:].index(tb + 1)
                    nc.gpsimd.collective_compute(
                        "AllGather", mybir.AluOpType.bypass, replica_groups=groups,
                        ins=[h0s_loc[_rbh[j]:_rbh[j + 1], :]],
                        outs=[h0s_ext[_rowoff_h[j]:_rowoff_h[j + 1], :]])

        # gather-phase constants (loaded after phase 1 so x tiles go first)
        iota_sb = load_const(iota, [128, 128], dt.float16)
        identh_sb = load_const(identh, [128, 128], dt.float16)
        ones_sb = load_const(ones, [128, 128], dt.float32)
        onesb_sb = load_const(onesb, [1, 128], dt.float16)
        w_sb = {n: load_const(a, [128, HC, D], dt.float16)
                for n, a in (("q", wq), ("k", wk), ("v", wv), ("s", ws))}
        idxa_sb = load_const(idxa, [128, G * TG * 8], dt.int16)
        idxkv_sb = load_const(idxkv, [128, G * TG * 8], dt.int16)
        sega_sb = load_const(sega, [128, G * TG], dt.float16)
        seg32_sb = load_const(seg32, [128, G * TG], dt.float32)

        b_sb = bgb_sb = None
        if not bias_zero:
            b_sb = {n: load_const(a, [1, D], dt.float16)
                    for n, a in (("q", bq), ("k", bk), ("v", bv), ("s", bs))}
            bg_sb = load_const(bg, [1, H], dt.float16)
            with tc.tile_pool(name="psb", bufs=1, space="PSUM") as psb:
                pb = psb.tile([128, H], dt.float32)
                nc.tensor.matmul(pb[:], lhsT=onesb_sb[:1, :], rhs=bg_sb[:1, :],
                                 start=True, stop=True)
                bgb_sb = cpool.tile([128, H], dt.float32)
                nc.vector.tensor_copy(bgb_sb[:], pb[:])

        # ============ sweep A: GCN aggregation + projections + kv exchange =========
        TG1 = min(8, TG)            # early tiles (srcs in h0s AG chunk 0)
        h0s_early = h0s_ext[0:_rowoff_h[1], :] if split_ok else h0s_ext
        with tc.tile_pool(name="gaA", bufs=11) as gaA_p, \
             tc.tile_pool(name="gaB", bufs=4) as gaB_p, \
             tc.tile_pool(name="sa", bufs=4) as sa_p, \
             tc.tile_pool(name="aps", bufs=2, space="PSUM") as aps, \
             tc.tile_pool(name="hsb", bufs=2) as hsb_p, \
             tc.tile_pool(name="ht", bufs=2) as ht_p, \
             tc.tile_pool(name="tps", bufs=2, space="PSUM") as tps, \
             tc.tile_pool(name="qps", bufs=4, space="PSUM") as qps, \
             tc.tile_pool(name="stg", bufs=2) as stg:
            gaA_t = []
            for g in range(G):
                gaA = gaA_p.tile([128, TG1, H], dt.float16)
                gather_tiles(gaA, h0s_early, idxa_sb, g, 0, TG1, H)
                gaA_t.append(gaA)
            for g in range(G):
                gaB = None
                if TG > TG1:
                    gaB = gaB_p.tile([128, TG - TG1, H], dt.float16)
                    gather_tiles(gaB, h0s_ext, idxa_sb, g, TG1, TG, H)
                gaA = gaA_t[g]
                sg = sa_p.tile([128, TG, 128], dt.float16)
                nc.vector.tensor_tensor(
                    out=sg[:],
                    in0=iota_sb[:].unsqueeze(1).broadcast_to([128, TG, 128]),
                    in1=sega_sb[:, g * TG:(g + 1) * TG].unsqueeze(2)
                        .broadcast_to([128, TG, 128]),
                    op=OP.is_equal)
                ph = aps.tile([128, H], dt.float32)
                for t in range(TG):
                    ga_ap = gaA[:, t, :] if t < TG1 else gaB[:, t - TG1, :]
                    nc.tensor.matmul(ph[:], lhsT=sg[:, t, :], rhs=ga_ap,
                                     start=(t == 0), stop=False)
                # + self-loop term via identity matmul of own h0s rows
                nc.tensor.matmul(ph[:], lhsT=identh_sb[:], rhs=h0s_own[:, g, :],
                                 start=False, stop=True)
                h16 = hsb_p.tile([128, H], dt.float16)
                if bias_zero:
                    # LeakyReLU(dinv*x) == dinv*LeakyReLU(x), dinv > 0
                    nc.scalar.activation(h16[:], ph[:], AF.Lrelu,
                                         scale=dinv_sb[:, g:g + 1],
                                         alpha=LEAKY_SLOPE)
                else:
                    hf = hsb_p.tile([128, H], dt.float32, tag="hf")
                    nc.vector.tensor_scalar(out=hf[:], in0=ph[:],
                                            scalar1=dinv_sb[:, g:g + 1],
                                            scalar2=None, op0=OP.mult)
                    nc.vector.tensor_tensor(out=hf[:], in0=hf[:], in1=bgb_sb[:],
                                            op=OP.add)
                    nc.scalar.activation(h16[:], hf[:], AF.Lrelu, alpha=LEAKY_SLOPE)
                # ---- layer-2 projections for this group
                ht = ht_p.tile([128, HC, 128], dt.float16)
                for hc in range(HC):
                    pt = tps.tile([128, 128], dt.float16)
                    nc.tensor.transpose(pt[:], h16[:, hc * 128:(hc + 1) * 128],
                                        identh_sb[:])
                    eng = (nc.vector, nc.scalar)[hc % 2]
                    if hc % 2 == 0:
                        nc.vector.tensor_copy(ht[:, hc, :], pt[:])
                    else:
                        nc.scalar.activation(ht[:, hc, :], pt[:], AF.Copy)
                kv_st = stg.tile([128, 2, D], dt.float16, tag="kv_st")
                for i, name in enumerate(("q", "k", "v", "s")):
                    pq = qps.tile([128, D], dt.float32)
                    for hc in range(HC):
                        last = (hc == HC - 1) and bias_zero
                        nc.tensor.matmul(pq[:], lhsT=ht[:, hc, :],
                                         rhs=w_sb[name][:, hc, :],
                                         start=(hc == 0), stop=last)
                    if not bias_zero:
                        nc.tensor.matmul(pq[:], lhsT=onesb_sb[:1, :],
                                         rhs=b_sb[name][:1, :],
                                         start=False, stop=True)
                    dst_ap = {"q": q_all[:, g, :], "k": kv_st[:, 0, :],
                              "v": kv_st[:, 1, :], "s": s_all[:, g, :]}[name]
                    if i % 2 == 0:
                        nc.vector.tensor_copy(dst_ap, pq[:])
                    else:
                        nc.scalar.activation(dst_ap, pq[:], AF.Copy)
                nc.sync.dma_start(out=kv_local[g * 128:(g + 1) * 128, :],
                                  in_=kv_st[:].rearrange("p a b -> p (a b)"))
                if (g + 1) in GBK[1:]:
                    j = GBK[1:].index(g + 1)
                    nc.gpsimd.collective_compute(
                        "AllGather", mybir.AluOpType.bypass, replica_groups=groups,
                        ins=[kv_local[_rb[j]:_rb[j + 1], :]],
                        outs=[kv_full[_rowoff[j]:_rowoff[j + 1], :]])

        # ================= sweep B: alpha ==========================================
        kv_early = kv_full[0:_rowoff[2], :] if split_ok else kv_full
        with tc.tile_pool(name="kgA", bufs=11) as kgA_p, \
             tc.tile_pool(name="kgB", bufs=4) as kgB_p, \
             tc.tile_pool(name="stt", bufs=4) as st_p, \
             tc.tile_pool(name="bps", bufs=6, space="PSUM") as bps, \
             tc.tile_pool(name="scr", bufs=6) as scr_p:
            kgA_t = []
            for g in range(G):
                kgA = kgA_p.tile([128, TG1, 2 * D], dt.float16)
                gather_tiles(kgA, kv_early, idxkv_sb, g, 0, TG1, 2 * D)
                kgA_t.append(kgA)
            for g in range(G):
                kgB = None
                if TG > TG1:
                    kgB = kgB_p.tile([128, TG - TG1, 2 * D], dt.float16)
                    gather_tiles(kgB, kv_full, idxkv_sb, g, TG1, TG, 2 * D)
                kgA = kgA_t[g]
                stg_t = st_p.tile([128, TG, 128], dt.float16)
                nc.sync.dma_start(
                    out=stg_t[:],
                    in_=st_in[:, g * TG * 128:(g + 1) * TG * 128].rearrange(
                        "p (t e) -> p t e", t=TG))
                if g % 2 == 0:
                    nc.vector.tensor_copy(vkeep[:, g * TG:g * TG + TG1, :],
                                          kgA[:, :, D:2 * D])
                    if kgB is not None:
                        nc.vector.tensor_copy(vkeep[:, g * TG + TG1:(g + 1) * TG, :],
                                              kgB[:, :, D:2 * D])
                else:
                    nc.scalar.activation(vkeep[:, g * TG:g * TG + TG1, :],
                                         kgA[:, :, D:2 * D], AF.Copy)
                    if kgB is not None:
                        nc.scalar.activation(vkeep[:, g * TG + TG1:(g + 1) * TG, :],
                                             kgB[:, :, D:2 * D], AF.Copy)
                for t in range(TG):
                    kg_ap = kgA[:, t, 0:D] if t < TG1 else kgB[:, t - TG1, 0:D]
                    pq = bps.tile([128, D], dt.float32)
                    nc.tensor.matmul(pq[:], lhsT=stg_t[:, t, :], rhs=q_all[:, g, :],
                                     start=True, stop=True)
                    scr = scr_p.tile([128, D], dt.float16)
                    gt = g * TG + t
                    nc.vector.scalar_tensor_tensor(
                        out=scr[:], in0=pq[:], scalar=1.0, in1=kg_ap,
                        op0=OP.mult, op1=OP.mult,
                        accum_out=alpha_all[:, gt:gt + 1])

        # ================= stats + AllReduce + per-edge scale ======================
        with tc.tile_pool(name="stp", bufs=1) as stat_p, \
             tc.tile_pool(name="stps", bufs=2, space="PSUM") as stps:
            asq = stat_p.tile([128, G * TG], dt.float32)
            nc.vector.tensor_tensor(out=asq[:], in0=alpha_all[:], in1=alpha_all[:],
                                    op=OP.mult)
            st2 = stat_p.tile([128, 2], dt.float32)
            nc.vector.tensor_reduce(out=st2[:, 0:1], in_=alpha_all[:],
                                    axis=mybir.AxisListType.X, op=OP.add)
            nc.vector.tensor_reduce(out=st2[:, 1:2], in_=asq[:],
                                    axis=mybir.AxisListType.X, op=OP.add)
            ps1 = stps.tile([1, 2], dt.float32)
            nc.tensor.matmul(ps1[:], lhsT=ones_sb[:, 0:1], rhs=st2[:], start=True,
                             stop=True)
            ccs = stat_p.tile([1, 2], dt.float32)
            nc.vector.tensor_copy(ccs[:], ps1[:])
            nc.sync.dma_start(out=cc_in, in_=ccs[:])
            nc.gpsimd.collective_compute(
                "AllReduce", mybir.AluOpType.add, replica_groups=groups,
                ins=[cc_in], outs=[cc_out])
            ccr = stat_p.tile([1, 2], dt.float32)
            nc.sync.dma_start(out=ccr[:], in_=cc_out)
            # mu = S1/E ; var = (S2 - S1*mu)/(E-1) ; c = SCALE/sqrt(var)
            mu = stat_p.tile([1, 1], dt.float32)
            nc.vector.tensor_scalar(out=mu[:], in0=ccr[:, 0:1], scalar1=1.0 / E,
                                    scalar2=None, op0=OP.mult)
            var = stat_p.tile([1, 1], dt.float32)
            nc.vector.tensor_tensor(out=var[:], in0=ccr[:, 0:1], in1=mu[:], op=OP.mult)
            nc.vector.tensor_tensor(out=var[:], in0=ccr[:, 1:2], in1=var[:],
                                    op=OP.subtract)
            nc.vector.tensor_scalar(out=var[:], in0=var[:], scalar1=1.0 / (E - 1),
                                    scalar2=None, op0=OP.mult)
            nc.scalar.activation(var[:], var[:], AF.Sqrt)
            cfac = stat_p.tile([1, 1], dt.float32)
            nc.vector.reciprocal(cfac[:], var[:])
            nc.vector.tensor_scalar(out=cfac[:], in0=cfac[:],
                                    scalar1=float(SCALE_PARAM),
                                    scalar2=None, op0=OP.mult)
            mc = stat_p.tile([1, 2], dt.float32)
            nc.vector.tensor_copy(mc[:, 0:1], mu[:])
            nc.vector.tensor_copy(mc[:, 1:2], cfac[:])
            pb2 = stps.tile([128, 2], dt.float32)
            nc.tensor.matmul(pb2[:], lhsT=ones_sb[0:1, :], rhs=mc[:1, :], start=True,
                             stop=True)
            mc_col = stat_p.tile([128, 2], dt.float32)
            nc.vector.tensor_copy(mc_col[:], pb2[:])
            # scale = sigmoid((alpha - mu) * c)   (pad edges give garbage; killed by
            # the is_equal(seg=-1) indicator in sweep C)
            an = stat_p.tile([128, G * TG], dt.float32)
            nc.vector.tensor_scalar(out=an[:], in0=alpha_all[:],
                                    scalar1=mc_col[:, 0:1], scalar2=mc_col[:, 1:2],
                                    op0=OP.subtract, op1=OP.mult)
            nc.scalar.activation(an[:], an[:], AF.Sigmoid)
            scale16 = apool.tile([128, G * TG], dt.float16)
            nc.vector.tensor_copy(scale16[:], an[:])

        # ================= sweep C: output aggregation =============================
        # unscaled S streamed from host (e-part one-hot); per-edge scale folded
        # into v (per-partition scalar), split between Scalar and Vector engines.
        with tc.tile_pool(name="sc", bufs=3) as sc_p, \
             tc.tile_pool(name="vs", bufs=4) as vs_p, \
             tc.tile_pool(name="ops", bufs=2, space="PSUM") as ops, \
             tc.tile_pool(name="ot", bufs=2) as ot_p:
            for g in range(G):
                sg = sc_p.tile([128, TG, 128], dt.float16)
                nc.sync.dma_start(
                    out=sg[:],
                    in_=stc_in[:, g * TG * 128:(g + 1) * TG * 128].rearrange(
                        "p (t e) -> p t e", t=TG))
                vsc = vs_p.tile([128, TG, D], dt.float16)
                nc.vector.tensor_tensor(
                    out=vsc[:], in0=vkeep[:, g * TG:(g + 1) * TG, :],
                    in1=scale16[:, g * TG:(g + 1) * TG].unsqueeze(2)
                        .broadcast_to([128, TG, D]),
                    op=OP.mult)
                po = ops.tile([128, D], dt.float32)
                for t in range(TG):
                    nc.tensor.matmul(po[:], lhsT=sg[:, t, :], rhs=vsc[:, t, :],
                                     start=(t == 0), stop=False)
                # + root_weight skip via identity matmul
                nc.tensor.matmul(po[:], lhsT=identh_sb[:], rhs=s_all[:, g, :],
                                 start=False, stop=True)
                ot = ot_p.tile([128, D], dt.float32)
                if g % 2 == 0:
                    nc.vector.tensor_copy(ot[:], po[:])
                else:
                    nc.scalar.activation(ot[:], po[:], AF.Copy)
                nc.sync.dma_start(out=out_l[g * 128:(g + 1) * 128, :], in_=ot[:])

    nc.compile()
    return nc


# -------------------- driver -------------------------------------------------------

_CACHE = {}


def _get_program(cfg, TG, bias_zero, split_ok):
    key = (tuple(sorted((k, v) for k, v in cfg.items() if not isinstance(v, tuple))),
           cfg["GBK"], cfg["GBH"], TG, bias_zero, split_ok)
    if key not in _CACHE:
        _CACHE[key] = build_program(cfg, TG, bias_zero, split_ok)
    return _CACHE[key]


def run(inputs, cfg_base=None, trace=False):
    cfg = _derive(cfg_base or FULL_CFG)
    in_maps, dyn, unperm = prep_host(inputs, cfg)
    nc = _get_program(cfg, dyn["TG"], dyn["bias_zero"], dyn["split_ok"])
    from concourse.bass_utils import run_bass_kernel_spmd
    res = run_bass_kernel_spmd(nc, in_maps, list(range(cfg["C"])), trace=trace)
    full = np.concatenate([res.results[c]["out"] for c in range(cfg["C"])], 0)
    out = full[unperm["slot"]]
    return out.astype(np.float32), res


def kernel(**inputs):
    out, _ = run(inputs)
    return out
